# revision 2
# baseline (speedup 1.0000x reference)
"""DaGCN on 8 Trainium2 NeuronCores (Bass SPMD) — v2.

Changes vs v1 (1472us cost-model):
  * ONE merged AllGather for the L1 tables: t12 = [s1|s2] rows [SP, 256]
    bf16 -> out 25.7MB @ ~285us (vs 2x252us), exploiting the collective
    cost model's concave bandwidth ramp.
  * L2 table collective gathers the PACKED [SP, 64] bf16 shard (out 6.4MB
    @ ~176us vs 252us).  Gathers index NODE PAIRS (row//2, int16-safe for
    all 50176 rows) and select the even/odd node via the 256B-aligned
    gather offset; edges are split by src-row parity instead of lo/hi
    halves, so L1 and L2 share one set of idx/dst/ew arrays, loaded once.
  * Phase A computes the s-tables node-major directly (no transposes).
  * L2 segment-sum accumulates each dst block in a RESIDENT psum tile
    across both parity runs (one evacuation per block, on ACT).
  * Edge meta (gidx/dst/ew) stays SBUF-resident for all 4 passes.
"""

import math
from contextlib import ExitStack

import ml_dtypes
import numpy as np

import concourse.bacc as bacc
import concourse.bass as bass
import concourse.mybir as mybir
from concourse.bass_utils import run_bass_kernel_spmd

F32 = mybir.dt.float32
BF16 = mybir.dt.bfloat16
I16 = mybir.dt.int16
AOP = mybir.AluOpType
ACT = mybir.ActivationFunctionType

NCORES = 8
N = 50000
NFEAT, NHID, NCLASS = 256, 64, 32
S_CALL = 1024          # idxs per dma_gather call (HW-validated; 2048 hangs)
CALL_CHUNKS = S_CALL // 128
RING = 16              # gather/onehot ring depth (in calls)
NPSUM = 4              # psum block-accumulator ring (L1)


def _wrap16(a):
    """[n] int16 -> [128, n//16]: idx i at [i%16, i//16], replicated x8."""
    n = a.shape[0]
    w = a.reshape(n // 16, 16).T.astype(np.int16)
    return np.tile(w, (8, 1)).copy()


def _chunkwrap(a, dtype):
    """[n] -> [128, n//128]: edge i at [i%128, i//128]."""
    n = a.shape[0]
    return np.ascontiguousarray(a.reshape(n // 128, 128).T.astype(dtype))


def _prep_adjacency(src, dst, ew, S, SP, NB, NROWS):
    """Bucket edges by (dst core, src-row parity, dst block).

    Returns per-core (gidx, dcol, eww) arrays plus the shared compile-time
    schedule: cpb[p][b] = chunks for (parity p, block b), identical across
    cores (max), with per-parity chunk counts padded to CALL_CHUNKS.
    """
    src = np.asarray(src).astype(np.int64)
    dst = np.asarray(dst).astype(np.int64)
    ew = np.asarray(ew).astype(np.float32)
    core = dst // S
    row = (src // S) * SP + (src % S)       # padded table row
    par = row % 2
    pair = row // 2
    dstrel = dst - core * S
    blk = dstrel // 128
    col = dstrel % 128

    percore = []
    counts = np.zeros((NCORES, 2, NB), np.int64)
    for k in range(NCORES):
        m = core == k
        e = np.lexsort((blk[m], par[m]))   # sort by (parity, block)
        r, h, b, c, w = pair[m][e], par[m][e], blk[m][e], col[m][e], ew[m][e]
        percore.append((r, h, b, c, w))
        for p in range(2):
            mm = h == p
            counts[k, p] = np.bincount(b[mm], minlength=NB)

    cpb = np.maximum(np.ceil(counts.max(axis=0) / 128).astype(np.int64), 1)
    # parity-1 runs may be empty only if parity-0 handles init; keep >=1 on p0
    ch = [int(cpb[p].sum()) for p in range(2)]
    chp = [-(-c // CALL_CHUNKS) * CALL_CHUNKS for c in ch]
    # trailing pad chunks extend block NB-1's run of that parity
    nslot = (chp[0] + chp[1]) * 128

    # chunk offset of (p, b)
    coff = np.zeros((2, NB), np.int64)
    coff[0] = np.concatenate(([0], np.cumsum(cpb[0])))[:-1]
    coff[1] = chp[0] + np.concatenate(([0], np.cumsum(cpb[1])))[:-1]

    out = []
    for k in range(NCORES):
        r, h, b, c, w = percore[k]
        gidx = np.zeros(nslot, np.int64)
        dcol = np.zeros(nslot, np.int64)
        eww = np.zeros(nslot, np.float32)
        for p in range(2):
            mm = h == p
            rr, bb, cc, ww = r[mm], b[mm], c[mm], w[mm]
            cnt = counts[k, p]
            offs = np.concatenate(([0], np.cumsum(cnt)))[:-1]
            pos = np.arange(rr.shape[0]) - offs[bb]
            slot = (coff[p][bb]) * 128 + pos
            gidx[slot] = rr
            dcol[slot] = cc
            eww[slot] = ww
        out.append((
            _wrap16(gidx),
            _chunkwrap(dcol, np.float32),
            _chunkwrap(eww, np.float32),
        ))
    return out, (cpb[0].tolist(), cpb[1].tolist(), chp[0], chp[1], nslot)


def _sched_chunks(meta, NB):
    """Per global chunk: (parity, block, start, stop). Pads extend the last
    block's run of their parity."""
    cpb0, cpb1, chp0, chp1, _ = meta
    sched = []
    for p, cpbs, chp in ((0, cpb0, chp0), (1, cpb1, chp1)):
        for b in range(NB):
            for i in range(cpbs[b]):
                sched.append([p, b, i == 0, False])
        for _ in range(chp - sum(cpbs)):
            sched.append([p, NB - 1, False, False])
    # mark stops: last chunk of each (p, b)
    last = {}
    for i, (p, b, st, sp) in enumerate(sched):
        last[(p, b)] = i
    for (p, b), i in last.items():
        sched[i][3] = True
    return sched


class Ctr:
    def __init__(self, sem, step=1):
        self.sem, self.n, self.step = sem, 0, step

    def inc(self, inst):
        inst.then_inc(self.sem, self.step)
        self.n += self.step
        return self.n


def _build(S, SP, NB, NROWS, adjmeta, scalars):
    nc = bacc.Bacc("TRN2", num_devices=NCORES, num_swdge_queues=2)
    g1b, g2b, h1b, h2b = scalars
    NPAIR = NROWS // 2

    # ---------------- I/O ----------------
    din = {}
    for v in ("xt1a", "xt1b", "xt2a", "xt2b"):
        din[v] = nc.dram_tensor(v, [128, 2, SP], BF16, kind="ExternalInput")
    din["w1a"] = nc.dram_tensor("w1a", [128, 2, NHID], BF16, kind="ExternalInput")
    din["w1b"] = nc.dram_tensor("w1b", [128, 2, NHID], BF16, kind="ExternalInput")
    din["w2"] = nc.dram_tensor("w2", [128, 64], BF16, kind="ExternalInput")
    din["iota"] = nc.dram_tensor("iota", [128, 128], BF16, kind="ExternalInput")
    din["idf"] = nc.dram_tensor("idf", [128, 128], BF16, kind="ExternalInput")
    din["g1w"] = nc.dram_tensor("g1w", [128, 128], BF16, kind="ExternalInput")
    din["g2w"] = nc.dram_tensor("g2w", [128, 128], BF16, kind="ExternalInput")
    din["h1w"] = nc.dram_tensor("h1w", [128, 64], BF16, kind="ExternalInput")
    din["h2w"] = nc.dram_tensor("h2w", [128, 64], BF16, kind="ExternalInput")
    din["b1r"] = nc.dram_tensor("b1r", [128, 128], BF16, kind="ExternalInput")
    din["b2r"] = nc.dram_tensor("b2r", [128, 64], BF16, kind="ExternalInput")
    for a in (1, 2):
        ns = adjmeta[a][4]
        din[f"gidx{a}"] = nc.dram_tensor(f"gidx{a}", [128, ns // 16], I16, kind="ExternalInput")
        din[f"dst{a}"] = nc.dram_tensor(f"dst{a}", [128, ns // 128], F32, kind="ExternalInput")
        din[f"eww{a}"] = nc.dram_tensor(f"eww{a}", [128, ns // 128], F32, kind="ExternalInput")
    out_o = nc.dram_tensor("out_o", [SP, NCLASS], BF16, kind="ExternalOutput")
    p1_o = nc.dram_tensor("p1_o", [SP, NCLASS], BF16, kind="ExternalOutput")
    p2_o = nc.dram_tensor("p2_o", [SP, NCLASS], BF16, kind="ExternalOutput")

    t12_in = nc.dram_tensor("t12in", [SP, 256], BF16)
    t12_full = nc.dram_tensor("t12full", [NROWS, 256], BF16, addr_space="Shared")
    t3_in = nc.dram_tensor("t3in", [SP, NHID], BF16)
    # one pad row so the odd-parity strided gather view stays in bounds
    t3_full = nc.dram_tensor("t3full", [NPAIR + 1, 128], BF16, addr_space="Shared")

    ctx = ExitStack()
    sb = lambda name, shape, dt: ctx.enter_context(nc.sbuf_tensor(name, shape, dt))
    ps = lambda name, shape: ctx.enter_context(nc.psum_tensor(name, shape, F32))
    sem = lambda name: ctx.enter_context(nc.semaphore(name))

    # ---------------- SBUF ----------------
    c_w1a = sb("c_w1a", [128, 2, NHID], BF16)
    c_w1b = sb("c_w1b", [128, 2, NHID], BF16)
    c_w2 = sb("c_w2", [128, 64], BF16)
    c_iota = sb("c_iota", [128, 128], BF16)
    c_idf = sb("c_idf", [128, 128], BF16)
    c_g1w = sb("c_g1w", [128, 128], BF16)
    c_g2w = sb("c_g2w", [128, 128], BF16)
    c_h1w = sb("c_h1w", [128, 64], BF16)
    c_h2w = sb("c_h2w", [128, 64], BF16)
    c_b1r = sb("c_b1r", [128, 128], BF16)
    c_b2r = sb("c_b2r", [128, 64], BF16)
    cbias = sb("cbias", [128, 4], F32)

    tstage = sb("tstage", [128, NB, 256], BF16)
    ns1 = adjmeta[1][4]
    ns2 = adjmeta[2][4]
    gidx_sb = {1: sb("gidx1_sb", [128, ns1 // 16], I16),
               2: sb("gidx2_sb", [128, ns2 // 16], I16)}
    dst_sb = {1: sb("dst1_sb", [128, ns1 // 128], F32),
              2: sb("dst2_sb", [128, ns2 // 128], F32)}
    ew_sb = {1: sb("ew1_sb", [128, ns1 // 128], F32),
             2: sb("ew2_sb", [128, ns2 // 128], F32)}
    lamv = {nm: sb(nm, [128, NB], F32)
            for nm in ("l1", "l2", "lsum", "w0", "w1")}
    wbf = {nm: sb(f"wb_{nm}", [128, NB], BF16) for nm in ("w0", "w1")}
    xtt = sb("xtt", [128, 128], BF16)

    sbA = ExitStack()
    xts = {v: sbA.enter_context(nc.sbuf_tensor(f"x{v}", [128, 2, SP], BF16))
           for v in ("xt1a", "xt1b", "xt2a", "xt2b")}

    psA = ExitStack()
    mm_ps = [psA.enter_context(nc.psum_tensor(f"mm_ps{i}", [128, 256], F32))
             for i in range(2)]

    io = Ctr(sem("io"), 16)        # sync-engine DMAs
    gsems = [Ctr(sem(f"g{i}"), 16) for i in range(RING)]
    ccs = [Ctr(sem(f"cc{i}"), 1) for i in range(2)]
    pe = Ctr(sem("pe"), 1)
    dv = Ctr(sem("dv"), 1)
    ac = Ctr(sem("ac"), 1)
    gp = Ctr(sem("gp"), 1)

    SY, PE, DV, AC, GP = nc.sync, nc.tensor, nc.vector, nc.scalar, nc.gpsimd

    def fence():
        SY.wait_ge(io.sem, io.n)

    # =========== Phase A: constants + node-major s tables ===========
    for bi, bval in enumerate((g1b, g2b, h1b, h2b)):
        nc.vector.memset(cbias[:, bi:bi + 1], float(bval))
    dv.inc(DV.memset(xtt[:], 0))
    SY.wait_ge(dv.sem, dv.n)
    io.inc(SY.dma_start(t3_full[NPAIR:NPAIR + 1, :], xtt[0:1, :]))
    for name, t in (("w1a", c_w1a), ("w1b", c_w1b), ("w2", c_w2), ("iota", c_iota),
                    ("idf", c_idf), ("g1w", c_g1w), ("g2w", c_g2w),
                    ("h1w", c_h1w), ("h2w", c_h2w), ("b1r", c_b1r), ("b2r", c_b2r)):
        io.inc(SY.dma_start(t[:], din[name][:]))
    HB = NB // 2 + 1          # node-slice halves for load/compute overlap
    halves = [(0, min(HB * 128, SP))]
    if HB * 128 < SP:
        halves.append((HB * 128, SP))
    xload = {}
    for vi, (va, vb) in enumerate((("xt1a", "xt1b"), ("xt2a", "xt2b"))):
        for hi, (o0, o1) in enumerate(halves):
            io.inc(SY.dma_start(xts[va][:, :, o0:o1], din[va][:, :, o0:o1]))
            io.inc(SY.dma_start(xts[vb][:, :, o0:o1], din[vb][:, :, o0:o1]))
            fence()
            xload[(vi, hi)] = io.n
        if len(halves) == 1:
            xload[(vi, 1)] = io.n

    # t12 row n = [s1a[n] | s1b[n] | s2a[n] | s2b[n]], 256 bf16 cols
    stc = {}
    nmm = 0
    for vi, (va, vb) in enumerate((("xt1a", "xt1b"), ("xt2a", "xt2b"))):
        for j in range(NB):
            if j == 0:
                PE.wait_ge(io.sem, xload[(vi, 0)])
            if j == HB:
                PE.wait_ge(io.sem, xload[(vi, 1)])
            p = mm_ps[nmm % 2]
            if nmm >= 2:
                PE.wait_ge(dv.sem, stc[nmm - 2])
            o = j * 128
            last = None
            for bi, (xv, w) in enumerate(((va, c_w1a), (vb, c_w1b))):
                base = bi * 64
                for cch in range(2):
                    last = PE.matmul(p[:, base:base + 64],
                                     xts[xv][:, cch, o:o + 128],
                                     w[:, cch, :],
                                     start=(cch == 0), stop=(cch == 1))
            pe.inc(last)
            DV.wait_ge(pe.sem, pe.n)
            dv.inc(DV.tensor_copy(tstage[:, j, vi * 128:vi * 128 + 128],
                                  p[:, 0:128]))
            stc[nmm] = dv.n
            nmm += 1
    pe_phaseA = pe.n
    SY.wait_ge(dv.sem, dv.n)
    io.inc(SY.dma_start(
        t12_in[:].rearrange("(t p) f -> p t f", p=128), tstage[:]))
    fence()
    GP.wait_ge(io.sem, io.n)
    ccs[0].inc(GP.collective_compute(
        "AllGather", AOP.bypass, replica_groups=[list(range(NCORES))],
        ins=[t12_in[:]], outs=[t12_full[:]]))
    # idx loads hide under the collective
    for a in (1, 2):
        io.inc(SY.dma_start(gidx_sb[a][:], din[f"gidx{a}"][:]))
        io.inc(SY.dma_start(dst_sb[a][:], din[f"dst{a}"][:]))
        io.inc(SY.dma_start(ew_sb[a][:], din[f"eww{a}"][:]))
    fence()
    idx_io = io.n

    # =========== edge pass machinery ===========
    psA.close()
    sbA.close()
    psL1 = ExitStack()
    blk_ps = [psL1.enter_context(nc.psum_tensor(f"blk_ps{i}", [128, 128], F32))
              for i in range(NPSUM)]
    tr_ps = [psL1.enter_context(nc.psum_tensor(f"tr_ps{i}", [128, 64], F32))
             for i in range(2)]
    trb_ps = [psL1.enter_context(nc.psum_tensor(f"trb_ps{i}", [128, 128], BF16))
              for i in range(2)]
    msg = sb("msg", [128, RING * CALL_CHUNKS, 128], BF16)
    ohr = sb("ohr", [128, RING * CALL_CHUNKS, 128], BF16)
    agg1 = sb("agg1", [128, NB, 128], BF16)
    agg2 = sb("agg2", [128, NB, 128], BF16)
    tmp = sb("tmp", [128, NB, 128], BF16)
    tmp2 = sb("tmp2", [128, NB, 128], BF16)
    # L2 outputs alias agg2 (free after the gated combine); scratch aliases agg1
    prop1 = lambda b=None: agg2[:, :, 0:64] if b is None else agg2[:, b, 0:64]
    prop2 = lambda b=None: agg2[:, :, 64:128] if b is None else agg2[:, b, 64:128]
    t64f = lambda b=None: agg1[:, :, 0:64] if b is None else agg1[:, b, 0:64]

    scheds = {a: _sched_chunks(adjmeta[a], NB) for a in (1, 2)}
    gcall = [0]
    pe_cons_vals = []
    npass = [0]
    psum_last = {}   # psum key -> (eng, val) of evac freeing it

    # gather views
    v12 = t12_full[:].rearrange("(a b) f -> a (b f)", b=2)      # [NPAIR, 512]
    t3flat = t3_full[:].rearrange("a f -> (a f)")
    l2v = {0: t3_full[0:NPAIR, :],
           1: t3flat[64:64 + NPAIR * 128].rearrange("(a f) -> a f", f=128)}

    def edge_pass(adj, layer, dests, fold_eng_gp=False):
        """dests = (dest(b), scr(b), dest_full, scr_full)."""
        fold_eng = GP if fold_eng_gp else DV
        meta = adjmeta[adj]
        sched = scheds[adj]
        cpb0, cpb1, chp0, chp1, ns = meta
        if layer == 1:
            inap = {p: v12[:, (adj - 1) * 128 + p * 256:
                           (adj - 1) * 128 + p * 256 + 128] for p in (0, 1)}
            step = 512
        else:
            inap = l2v
            step = 128
        F = 128 if layer == 1 else 64
        dest, scr, dest_full, scr_full = dests
        first = npass[0] == 0
        npass[0] += 1
        cc_need = 0 if layer == 1 else 1
        GP.wait_ge(ccs[cc_need].sem, 1)
        if first:
            # msg/ohr alias freed phase-A space; DVE onehots must not race
            # phase-A PE reads (GP path is transitively safe via cc wait)
            DV.wait_ge(pe.sem, pe_phaseA)
        for p, chp in ((0, chp0), (1, chp1)):
            ch0 = 0 if p == 0 else chp0
            for j in range(chp // CALL_CHUNKS):
                rj = (gcall[0] % RING) * CALL_CHUNKS
                gslot = gcall[0] % RING
                cbase = ch0 + j * CALL_CHUNKS
                if gcall[0] == 0:
                    GP.wait_ge(io.sem, idx_io)
                if len(pe_cons_vals) >= RING:
                    GP.wait_ge(pe.sem, pe_cons_vals[-RING])
                g = GP.dma_gather(
                    msg[:, rj:rj + CALL_CHUNKS, :], inap[p],
                    gidx_sb[adj][:, cbase * 8:(cbase + CALL_CHUNKS) * 8],
                    S_CALL, S_CALL, 128, elem_step=step,
                    queue_num=gcall[0] % 2)
                gsems[gslot].inc(g)
                gv = gsems[gslot].n
                # onehot build
                if gcall[0] == 0:
                    DV.wait_ge(io.sem, idx_io)
                if len(pe_cons_vals) >= RING:
                    DV.wait_ge(pe.sem, pe_cons_vals[-RING])
                for c8 in range(CALL_CHUNKS):
                    ts = DV.tensor_scalar(
                        ohr[:, rj + c8, :], c_iota[:],
                        dst_sb[adj][:, cbase + c8:cbase + c8 + 1],
                        ew_sb[adj][:, cbase + c8:cbase + c8 + 1],
                        op0=AOP.is_equal, op1=AOP.mult)
                dv.inc(ts)
                ohv = dv.n
                # matmuls
                PE.wait_ge(gsems[gslot].sem, gv)
                PE.wait_ge(dv.sem, ohv)
                for c8 in range(CALL_CHUNKS):
                    pp, b, st, sp = sched[cbase + c8]
                    key = b % NPSUM
                    ptile = blk_ps[key][:, 0:F]
                    if st and key in psum_last:
                        eng, val = psum_last[key]
                        PE.wait_ge({"dv": dv.sem, "ac": ac.sem}[eng], val)
                    mmi = PE.matmul(ptile, ohr[:, rj + c8, :],
                                    msg[:, rj + c8, 0:F],
                                    start=st, stop=sp)
                    if sp:
                        pe.inc(mmi)
                        AC.wait_ge(pe.sem, pe.n)
                        cpi = AC.activation(dest(b) if pp == 0 else scr(b),
                                            ptile, ACT.Copy)
                        ac.inc(cpi)
                        psum_last[key] = ("ac", ac.n)
                if not sp:
                    pe.inc(mmi)
                pe_cons_vals.append(pe.n)
                gcall[0] += 1
        # fold the parity-1 partials into dest (one wide bf16 add)
        if fold_eng is DV:
            DV.wait_ge(ac.sem, ac.n)
            dv.inc(DV.tensor_tensor(dest_full, dest_full, scr_full, op=AOP.add))
        else:
            GP.wait_ge(ac.sem, ac.n)
            GP.tensor_tensor(dest_full, dest_full, scr_full, op=AOP.add)
            GP.drain()

    edge_pass(1, 1, (lambda b: agg1[:, b, :], lambda b: tmp[:, b, :],
                     agg1[:], tmp[:]))
    edge_pass(2, 1, (lambda b: agg2[:, b, :], lambda b: tmp2[:, b, :],
                     agg2[:], tmp2[:]), fold_eng_gp=True)

    # =========== Phase C: mid gating + L2 table ===========
    # branch 1 on DVE, branch 2 on GPSIMD (Pool idle here), all bf16
    b1b = c_b1r[:, None, :].broadcast_to([128, NB, 128])
    g1b_b = c_g1w[:, None, :].broadcast_to([128, NB, 128])
    g2b_b = c_g2w[:, None, :].broadcast_to([128, NB, 128])
    DV.drain()
    DV.tensor_tensor(agg1[:], agg1[:], b1b, op=AOP.add)
    DV.drain()
    DV.tensor_scalar(agg1[:], agg1[:], 0.0, None, op0=AOP.max)
    DV.drain()
    DV.tensor_tensor(tmp[:], agg1[:], g1b_b, op=AOP.mult)
    DV.drain()
    dv.inc(DV.tensor_reduce(lamv["l1"][:], tmp[:], axis=mybir.AxisListType.X,
                            op=AOP.add))
    l1_dv = dv.n
    GP.tensor_tensor(agg2[:], agg2[:], b1b, op=AOP.add)
    GP.drain()
    GP.tensor_scalar(agg2[:], agg2[:], 0.0, None, op0=AOP.max)
    GP.drain()
    gp.inc(GP.tensor_tensor(tmp2[:], agg2[:], g2b_b, op=AOP.mult))
    DV.wait_ge(gp.sem, gp.n)
    dv.inc(DV.tensor_reduce(lamv["l2"][:], tmp2[:], axis=mybir.AxisListType.X,
                            op=AOP.add))
    l2_dv = dv.n
    AC.wait_ge(dv.sem, l2_dv)
    AC.activation(lamv["l1"][:], lamv["l1"][:], ACT.Sigmoid, bias=cbias[:, 0:1])
    ac.inc(AC.activation(lamv["l2"][:], lamv["l2"][:], ACT.Sigmoid, bias=cbias[:, 1:2]))
    DV.wait_ge(ac.sem, ac.n)
    DV.tensor_tensor(lamv["lsum"][:], lamv["l1"][:], lamv["l2"][:], op=AOP.add)
    DV.drain()
    DV.tensor_scalar(lamv["lsum"][:], lamv["lsum"][:], 1e-12, None, op0=AOP.max)
    DV.drain()
    DV.reciprocal(lamv["lsum"][:], lamv["lsum"][:])
    DV.drain()
    DV.tensor_tensor(lamv["w0"][:], lamv["l1"][:], lamv["lsum"][:], op=AOP.mult)
    DV.tensor_tensor(lamv["w1"][:], lamv["l2"][:], lamv["lsum"][:], op=AOP.mult)
    DV.drain()
    DV.tensor_copy(wbf["w0"][:], lamv["w0"][:])
    dv.inc(DV.tensor_copy(wbf["w1"][:], lamv["w1"][:]))
    wb_dv = dv.n
    w0b = wbf["w0"][:, :, None].broadcast_to([128, NB, 128])
    w1b_ = wbf["w1"][:, :, None].broadcast_to([128, NB, 128])
    DV.drain()
    DV.tensor_tensor(agg1[:], agg1[:], w0b, op=AOP.mult)
    GP.wait_ge(dv.sem, wb_dv)
    gp.inc(GP.tensor_tensor(tmp2[:], agg2[:], w1b_, op=AOP.mult))
    DV.drain()
    DV.wait_ge(gp.sem, gp.n)
    dv.inc(DV.tensor_tensor(agg1[:], agg1[:], tmp2[:], op=AOP.add))  # x -> agg1
    xfin = dv.n

    # L2 table: s2 = x @ W2 (pad to 64 cols), node-major rows
    stc2 = {}
    for t in range(NB):
        pb = trb_ps[t % 2]
        pf = tr_ps[t % 2]
        if t == 0:
            PE.wait_ge(dv.sem, xfin)
        if t >= 2:
            PE.wait_ge(dv.sem, stc2[t - 2])
        pe.inc(PE.transpose(pb[:], agg1[:, t, :], c_idf[:]))
        DV.wait_ge(pe.sem, pe.n)
        dv.inc(DV.tensor_copy(xtt[:], pb[:]))
        PE.wait_ge(dv.sem, dv.n)
        pe.inc(PE.matmul(pf[:], xtt[:], c_w2[:], start=True, stop=True))
        DV.wait_ge(pe.sem, pe.n)
        dv.inc(DV.tensor_copy(tstage[:, t, 0:64], pf[:]))
        stc2[t] = dv.n
    SY.wait_ge(dv.sem, dv.n)
    io.inc(SY.dma_start(t3_in[:].rearrange("(t p) f -> p t f", p=128),
                        tstage[:, :, 0:64]))
    fence()
    GP.wait_ge(io.sem, io.n)
    ccs[1].inc(GP.collective_compute(
        "AllGather", AOP.bypass, replica_groups=[list(range(NCORES))],
        ins=[t3_in[:]],
        outs=[t3_full[0:NPAIR, :].rearrange("a (b f) -> (a b) f", b=2)]))

    # =========== L2 edge passes ===========
    edge_pass(1, 2, (lambda b: prop1(b), lambda b: tmp[:, b, 0:64],
                     prop1(), tmp[:, :, 0:64]))
    edge_pass(2, 2, (lambda b: prop2(b), lambda b: tmp2[:, b, 0:64],
                     prop2(), tmp2[:, :, 0:64]), fold_eng_gp=True)
    psL1.close()

    # =========== Phase F: final gating + outputs ===========
    b2b = c_b2r[:, None, 0:64].broadcast_to([128, NB, 64])
    h1b_b = c_h1w[:, None, :].broadcast_to([128, NB, 64])
    h2b_b = c_h2w[:, None, :].broadcast_to([128, NB, 64])
    DV.drain()
    DV.tensor_tensor(prop1(), prop1(), b2b, op=AOP.add)
    DV.drain()
    dv.inc(DV.tensor_tensor(t64f(), prop1(), h1b_b, op=AOP.mult))
    DV.drain()
    dv.inc(DV.tensor_reduce(lamv["l1"][:], t64f(), axis=mybir.AxisListType.X,
                            op=AOP.add))
    l1_dv = dv.n
    GP.tensor_tensor(prop2(), prop2(), b2b, op=AOP.add)
    GP.drain()
    gp.inc(GP.tensor_tensor(tmp2[:, :, 0:64], prop2(), h2b_b, op=AOP.mult))
    DV.wait_ge(gp.sem, gp.n)
    dv.inc(DV.tensor_reduce(lamv["l2"][:], tmp2[:, :, 0:64],
                            axis=mybir.AxisListType.X, op=AOP.add))
    l2_dv = dv.n
    SY.wait_ge(dv.sem, l2_dv)
    SY.wait_ge(gp.sem, gp.n)
    io.inc(SY.dma_start(p1_o[:].rearrange("(t p) f -> p t f", p=128),
                        agg2[:, :, 0:NCLASS]))
    io.inc(SY.dma_start(p2_o[:].rearrange("(t p) f -> p t f", p=128),
                        agg2[:, :, 64:64 + NCLASS]))
    fence()
    pout_io = io.n
    AC.wait_ge(dv.sem, l2_dv)
    AC.activation(lamv["l1"][:], lamv["l1"][:], ACT.Sigmoid, bias=cbias[:, 2:3])
    ac.inc(AC.activation(lamv["l2"][:], lamv["l2"][:], ACT.Sigmoid, bias=cbias[:, 3:4]))
    DV.wait_ge(ac.sem, ac.n)
    DV.tensor_tensor(lamv["lsum"][:], lamv["l1"][:], lamv["l2"][:], op=AOP.add)
    DV.drain()
    DV.tensor_scalar(lamv["lsum"][:], lamv["lsum"][:], 1e-12, None, op0=AOP.max)
    DV.drain()
    DV.reciprocal(lamv["lsum"][:], lamv["lsum"][:])
    DV.drain()
    DV.tensor_tensor(lamv["w0"][:], lamv["l1"][:], lamv["lsum"][:], op=AOP.mult)
    DV.tensor_tensor(lamv["w1"][:], lamv["l2"][:], lamv["lsum"][:], op=AOP.mult)
    DV.drain()
    DV.tensor_copy(wbf["w0"][:], lamv["w0"][:])
    dv.inc(DV.tensor_copy(wbf["w1"][:], lamv["w1"][:]))
    wb_dv = dv.n
    w0b6 = wbf["w0"][:, :, None].broadcast_to([128, NB, 64])
    w1b6 = wbf["w1"][:, :, None].broadcast_to([128, NB, 64])
    DV.drain()
    DV.wait_ge(io.sem, pout_io)  # don't clobber props mid-DMA
    DV.tensor_tensor(t64f(), prop1(), w0b6, op=AOP.mult)
    GP.wait_ge(dv.sem, wb_dv)
    GP.wait_ge(io.sem, pout_io)
    gp.inc(GP.tensor_tensor(tmp2[:, :, 0:64], prop2(), w1b6, op=AOP.mult))
    DV.drain()
    DV.wait_ge(gp.sem, gp.n)
    dv.inc(DV.tensor_tensor(t64f(), t64f(), tmp2[:, :, 0:64], op=AOP.add))
    SY.wait_ge(dv.sem, dv.n)
    io.inc(SY.dma_start(out_o[:].rearrange("(t p) f -> p t f", p=128),
                        agg1[:, :, 0:NCLASS]))
    SY.wait_ge(io.sem, io.n)

    nc.compile()
    ctx.close()
    return nc


def _run(inputs, sim=False):
    S = inputs["x1a"].shape[0] // NCORES
    NB = -(-S // 128)
    SP = NB * 128
    NROWS = NCORES * SP

    adj = {}
    adjmeta = {}
    for a in (1, 2):
        out, meta = _prep_adjacency(
            inputs[f"src{a}"], inputs[f"dst{a}"], inputs[f"ew{a}"],
            S, SP, NB, NROWS)
        adj[a] = out
        adjmeta[a] = meta

    scalars = (float(np.asarray(inputs["g1b"]).ravel()[0]),
               float(np.asarray(inputs["g2b"]).ravel()[0]),
               float(np.asarray(inputs["h1b"]).ravel()[0]),
               float(np.asarray(inputs["h2b"]).ravel()[0]))
    nc = _build(S, SP, NB, NROWS, adjmeta, scalars)

    bf = ml_dtypes.bfloat16
    f32 = np.float32

    def wfmt(w):  # [256, 64] -> [128, 2, 64] bf16
        return np.ascontiguousarray(
            np.asarray(w, f32).reshape(2, 128, NHID).transpose(1, 0, 2)).astype(bf)

    w2pad = np.zeros((128, 64), f32)
    w2pad[:, :NCLASS] = np.asarray(inputs["W2"], f32)
    iota = np.tile(np.arange(128, dtype=f32), (128, 1))
    ident = np.eye(128, dtype=f32)
    g1w = np.tile(np.asarray(inputs["g1w"], f32).ravel(), (128, 1))
    g2w = np.tile(np.asarray(inputs["g2w"], f32).ravel(), (128, 1))
    h1w = np.zeros((128, 64), f32)
    h1w[:, :NCLASS] = np.asarray(inputs["h1w"], f32).ravel()
    h2w = np.zeros((128, 64), f32)
    h2w[:, :NCLASS] = np.asarray(inputs["h2w"], f32).ravel()
    b1r = np.tile(np.concatenate([np.asarray(inputs["b1a"], f32).ravel(),
                                  np.asarray(inputs["b1b"], f32).ravel()]), (128, 1))
    b2r = np.zeros((128, 64), f32)
    b2r[:, :NCLASS] = np.asarray(inputs["b2"], f32).ravel()

    common = dict(
        w1a=wfmt(inputs["W1a"]), w1b=wfmt(inputs["W1b"]),
        w2=w2pad.astype(bf), iota=iota.astype(bf), idf=ident.astype(bf),
        g1w=g1w.astype(bf), g2w=g2w.astype(bf),
        h1w=h1w.astype(bf), h2w=h2w.astype(bf),
        b1r=b1r.astype(bf), b2r=b2r.astype(bf))

    def xfmt(x, k):  # shard k, pad, transpose -> [128, 2, SP] bf16
        xs = np.asarray(x, f32)[k * S:(k + 1) * S]
        xp = np.zeros((SP, NFEAT), f32)
        xp[:S] = xs
        xt = xp.T.reshape(2, 128, SP).transpose(1, 0, 2)
        return np.ascontiguousarray(xt).astype(bf)

    in_maps = []
    for k in range(NCORES):
        m = dict(common)
        for v, key in (("xt1a", "x1a"), ("xt1b", "x1b"),
                       ("xt2a", "x2a"), ("xt2b", "x2b")):
            m[v] = xfmt(inputs[key], k)
        for a in (1, 2):
            g, d, e = adj[a][k]
            m[f"gidx{a}"] = g
            m[f"dst{a}"] = d
            m[f"eww{a}"] = e
        in_maps.append(m)

    global LAST_EXEC_NS
    if sim:
        from concourse.bass_interp import MultiCoreSim
        msim = MultiCoreSim(nc, NCORES)
        for k in range(NCORES):
            for name, arr in in_maps[k].items():
                msim.cores[k].tensor(name)[:] = arr
        msim.simulate()
        results = [{nm: msim.cores[k].tensor(nm).copy()
                    for nm in ("out_o", "p1_o", "p2_o")} for k in range(NCORES)]
    else:
        import os
        r = run_bass_kernel_spmd(nc, in_maps, list(range(NCORES)))
        LAST_EXEC_NS = r.exec_time_ns
        results = r.results

    outs = []
    for nm in ("out_o", "p1_o", "p2_o"):
        outs.append(np.concatenate([results[k][nm][:S] for k in range(NCORES)],
                    axis=0).astype(np.float32))
    return tuple(outs)


LAST_EXEC_NS = None


def kernel(**inputs):
    return _run(inputs, sim=False)


# revision 4
# speedup vs baseline: 1.0213x; 1.0213x over previous
"""DaGCN on 8 Trainium2 NeuronCores (Bass SPMD) — v2.

Changes vs v1 (1472us cost-model):
  * ONE merged AllGather for the L1 tables: t12 = [s1|s2] rows [SP, 256]
    bf16 -> out 25.7MB @ ~285us (vs 2x252us), exploiting the collective
    cost model's concave bandwidth ramp.
  * L2 table collective gathers the PACKED [SP, 64] bf16 shard (out 6.4MB
    @ ~176us vs 252us).  Gathers index NODE PAIRS (row//2, int16-safe for
    all 50176 rows) and select the even/odd node via the 256B-aligned
    gather offset; edges are split by src-row parity instead of lo/hi
    halves, so L1 and L2 share one set of idx/dst/ew arrays, loaded once.
  * Phase A computes the s-tables node-major directly (no transposes),
    x loads split in halves so PE starts early; idx loads hide under the
    first collective.
  * PSUM evacuations go to ACT (both parity runs), folded with one wide
    bf16 DVE add per pass; gating math runs in bf16 split DVE/GPSIMD.
  * The L2-table transpose+matmul chain is batched 7 dst-blocks per psum
    tile so cross-engine semaphore latency amortizes (30us -> ~10us).
  * Edge meta (gidx/dst/ew) stays SBUF-resident for all 4 passes.
"""

import math
from contextlib import ExitStack

import ml_dtypes
import numpy as np

import concourse.bacc as bacc
import concourse.bass as bass
import concourse.mybir as mybir
from concourse.bass_utils import run_bass_kernel_spmd

F32 = mybir.dt.float32
BF16 = mybir.dt.bfloat16
I16 = mybir.dt.int16
AOP = mybir.AluOpType
ACT = mybir.ActivationFunctionType

NCORES = 8
N = 50000
NFEAT, NHID, NCLASS = 256, 64, 32
S_CALL = 1024          # idxs per dma_gather call (HW-validated; 2048 hangs)
CALL_CHUNKS = S_CALL // 128
RING = 16              # gather/onehot ring depth (in calls)
NPSUM = 4              # psum block-accumulator ring (L1)


def _wrap16(a):
    """[n] int16 -> [128, n//16]: idx i at [i%16, i//16], replicated x8."""
    n = a.shape[0]
    w = a.reshape(n // 16, 16).T.astype(np.int16)
    return np.tile(w, (8, 1)).copy()


def _chunkwrap(a, dtype):
    """[n] -> [128, n//128]: edge i at [i%128, i//128]."""
    n = a.shape[0]
    return np.ascontiguousarray(a.reshape(n // 128, 128).T.astype(dtype))


def _prep_adjacency(src, dst, ew, S, SP, NB, NROWS):
    """Bucket edges by (dst core, src-row parity, dst block).

    Returns per-core (gidx, dcol, eww) arrays plus the shared compile-time
    schedule: cpb[p][b] = chunks for (parity p, block b), identical across
    cores (max), with per-parity chunk counts padded to CALL_CHUNKS.
    """
    src = np.asarray(src).astype(np.int64)
    dst = np.asarray(dst).astype(np.int64)
    ew = np.asarray(ew).astype(np.float32)
    core = dst // S
    row = (src // S) * SP + (src % S)       # padded table row
    par = row % 2
    pair = row // 2
    dstrel = dst - core * S
    blk = dstrel // 128
    col = dstrel % 128

    percore = []
    counts = np.zeros((NCORES, 2, NB), np.int64)
    for k in range(NCORES):
        m = core == k
        e = np.lexsort((blk[m], par[m]))   # sort by (parity, block)
        r, h, b, c, w = pair[m][e], par[m][e], blk[m][e], col[m][e], ew[m][e]
        percore.append((r, h, b, c, w))
        for p in range(2):
            mm = h == p
            counts[k, p] = np.bincount(b[mm], minlength=NB)

    cpb = np.maximum(np.ceil(counts.max(axis=0) / 128).astype(np.int64), 1)
    # parity-1 runs may be empty only if parity-0 handles init; keep >=1 on p0
    ch = [int(cpb[p].sum()) for p in range(2)]
    chp = [-(-c // CALL_CHUNKS) * CALL_CHUNKS for c in ch]
    # trailing pad chunks extend block NB-1's run of that parity
    nslot = (chp[0] + chp[1]) * 128

    # chunk offset of (p, b)
    coff = np.zeros((2, NB), np.int64)
    coff[0] = np.concatenate(([0], np.cumsum(cpb[0])))[:-1]
    coff[1] = chp[0] + np.concatenate(([0], np.cumsum(cpb[1])))[:-1]

    out = []
    for k in range(NCORES):
        r, h, b, c, w = percore[k]
        gidx = np.zeros(nslot, np.int64)
        dcol = np.zeros(nslot, np.int64)
        eww = np.zeros(nslot, np.float32)
        for p in range(2):
            mm = h == p
            rr, bb, cc, ww = r[mm], b[mm], c[mm], w[mm]
            cnt = counts[k, p]
            offs = np.concatenate(([0], np.cumsum(cnt)))[:-1]
            pos = np.arange(rr.shape[0]) - offs[bb]
            slot = (coff[p][bb]) * 128 + pos
            gidx[slot] = rr
            dcol[slot] = cc
            eww[slot] = ww
        out.append((
            _wrap16(gidx),
            _chunkwrap(dcol, np.float32),
            _chunkwrap(eww, np.float32),
        ))
    return out, (cpb[0].tolist(), cpb[1].tolist(), chp[0], chp[1], nslot)


def _sched_chunks(meta, NB):
    """Per global chunk: (parity, block, start, stop). Pads extend the last
    block's run of their parity."""
    cpb0, cpb1, chp0, chp1, _ = meta
    sched = []
    for p, cpbs, chp in ((0, cpb0, chp0), (1, cpb1, chp1)):
        for b in range(NB):
            for i in range(cpbs[b]):
                sched.append([p, b, i == 0, False])
        for _ in range(chp - sum(cpbs)):
            sched.append([p, NB - 1, False, False])
    # mark stops: last chunk of each (p, b)
    last = {}
    for i, (p, b, st, sp) in enumerate(sched):
        last[(p, b)] = i
    for (p, b), i in last.items():
        sched[i][3] = True
    return sched


class Ctr:
    def __init__(self, sem, step=1):
        self.sem, self.n, self.step = sem, 0, step

    def inc(self, inst):
        inst.then_inc(self.sem, self.step)
        self.n += self.step
        return self.n


def _build(S, SP, NB, NROWS, adjmeta, scalars):
    nc = bacc.Bacc("TRN2", num_devices=NCORES, num_swdge_queues=2)
    g1b, g2b, h1b, h2b = scalars
    NPAIR = NROWS // 2

    # ---------------- I/O ----------------
    din = {}
    for v in ("xt1a", "xt1b", "xt2a", "xt2b"):
        din[v] = nc.dram_tensor(v, [128, 2, SP], BF16, kind="ExternalInput")
    din["w1a"] = nc.dram_tensor("w1a", [128, 2, NHID], BF16, kind="ExternalInput")
    din["w1b"] = nc.dram_tensor("w1b", [128, 2, NHID], BF16, kind="ExternalInput")
    din["w2"] = nc.dram_tensor("w2", [128, 64], BF16, kind="ExternalInput")
    din["iota"] = nc.dram_tensor("iota", [128, 128], BF16, kind="ExternalInput")
    din["idf"] = nc.dram_tensor("idf", [128, 128], BF16, kind="ExternalInput")
    din["g1w"] = nc.dram_tensor("g1w", [128, 128], BF16, kind="ExternalInput")
    din["g2w"] = nc.dram_tensor("g2w", [128, 128], BF16, kind="ExternalInput")
    din["h1w"] = nc.dram_tensor("h1w", [128, 64], BF16, kind="ExternalInput")
    din["h2w"] = nc.dram_tensor("h2w", [128, 64], BF16, kind="ExternalInput")
    din["b1r"] = nc.dram_tensor("b1r", [128, 128], BF16, kind="ExternalInput")
    din["b2r"] = nc.dram_tensor("b2r", [128, 64], BF16, kind="ExternalInput")
    for a in (1, 2):
        ns = adjmeta[a][4]
        din[f"gidx{a}"] = nc.dram_tensor(f"gidx{a}", [128, ns // 16], I16, kind="ExternalInput")
        din[f"dst{a}"] = nc.dram_tensor(f"dst{a}", [128, ns // 128], F32, kind="ExternalInput")
        din[f"eww{a}"] = nc.dram_tensor(f"eww{a}", [128, ns // 128], F32, kind="ExternalInput")
    out_o = nc.dram_tensor("out_o", [SP, NCLASS], BF16, kind="ExternalOutput")
    p1_o = nc.dram_tensor("p1_o", [SP, NCLASS], BF16, kind="ExternalOutput")
    p2_o = nc.dram_tensor("p2_o", [SP, NCLASS], BF16, kind="ExternalOutput")

    t12_in = nc.dram_tensor("t12in", [SP, 256], BF16)
    t12_full = nc.dram_tensor("t12full", [NROWS, 256], BF16, addr_space="Shared")
    t3_in = nc.dram_tensor("t3in", [SP, NHID], BF16)
    # one pad row so the odd-parity strided gather view stays in bounds
    t3_full = nc.dram_tensor("t3full", [NPAIR + 1, 128], BF16, addr_space="Shared")

    ctx = ExitStack()
    sb = lambda name, shape, dt: ctx.enter_context(nc.sbuf_tensor(name, shape, dt))
    ps = lambda name, shape: ctx.enter_context(nc.psum_tensor(name, shape, F32))
    sem = lambda name: ctx.enter_context(nc.semaphore(name))

    # ---------------- SBUF ----------------
    c_w1a = sb("c_w1a", [128, 2, NHID], BF16)
    c_w1b = sb("c_w1b", [128, 2, NHID], BF16)
    c_w2 = sb("c_w2", [128, 64], BF16)
    c_iota = sb("c_iota", [128, 128], BF16)
    c_idf = sb("c_idf", [128, 128], BF16)
    c_g1w = sb("c_g1w", [128, 128], BF16)
    c_g2w = sb("c_g2w", [128, 128], BF16)
    c_h1w = sb("c_h1w", [128, 64], BF16)
    c_h2w = sb("c_h2w", [128, 64], BF16)
    c_b1r = sb("c_b1r", [128, 128], BF16)
    c_b2r = sb("c_b2r", [128, 64], BF16)
    cbias = sb("cbias", [128, 4], F32)

    tstage = sb("tstage", [128, NB, 256], BF16)
    ns1 = adjmeta[1][4]
    ns2 = adjmeta[2][4]
    gidx_sb = {1: sb("gidx1_sb", [128, ns1 // 16], I16),
               2: sb("gidx2_sb", [128, ns2 // 16], I16)}
    dst_sb = {1: sb("dst1_sb", [128, ns1 // 128], F32),
              2: sb("dst2_sb", [128, ns2 // 128], F32)}
    ew_sb = {1: sb("ew1_sb", [128, ns1 // 128], F32),
             2: sb("ew2_sb", [128, ns2 // 128], F32)}
    lamv = {nm: sb(nm, [128, NB], F32)
            for nm in ("l1", "l2", "lsum", "w0", "w1")}
    wbf = {nm: sb(f"wb_{nm}", [128, NB], BF16) for nm in ("w0", "w1")}
    xtt = sb("xtt", [128, 128], BF16)
    xttg = [sb(f"xttg{i}", [128, 896], BF16) for i in range(2)]

    sbA = ExitStack()
    xts = {v: sbA.enter_context(nc.sbuf_tensor(f"x{v}", [128, 2, SP], BF16))
           for v in ("xt1a", "xt1b", "xt2a", "xt2b")}

    psA = ExitStack()
    mm_ps = [psA.enter_context(nc.psum_tensor(f"mm_ps{i}", [128, 256], F32))
             for i in range(2)]

    io = Ctr(sem("io"), 16)        # sync-engine DMAs
    gsems = [Ctr(sem(f"g{i}"), 16) for i in range(RING)]
    ccs = [Ctr(sem(f"cc{i}"), 1) for i in range(2)]
    pe = Ctr(sem("pe"), 1)
    dv = Ctr(sem("dv"), 1)
    ac = Ctr(sem("ac"), 1)
    gp = Ctr(sem("gp"), 1)

    SY, PE, DV, AC, GP = nc.sync, nc.tensor, nc.vector, nc.scalar, nc.gpsimd

    def fence():
        SY.wait_ge(io.sem, io.n)

    # =========== Phase A: constants + node-major s tables ===========
    for bi, bval in enumerate((g1b, g2b, h1b, h2b)):
        nc.vector.memset(cbias[:, bi:bi + 1], float(bval))
    dv.inc(DV.memset(xtt[:], 0))
    SY.wait_ge(dv.sem, dv.n)
    io.inc(SY.dma_start(t3_full[NPAIR:NPAIR + 1, :], xtt[0:1, :]))
    for name, t in (("w1a", c_w1a), ("w1b", c_w1b), ("w2", c_w2), ("iota", c_iota),
                    ("idf", c_idf), ("g1w", c_g1w), ("g2w", c_g2w),
                    ("h1w", c_h1w), ("h2w", c_h2w), ("b1r", c_b1r), ("b2r", c_b2r)):
        io.inc(SY.dma_start(t[:], din[name][:]))
    HB = NB // 2 + 1          # node-slice halves for load/compute overlap
    halves = [(0, min(HB * 128, SP))]
    if HB * 128 < SP:
        halves.append((HB * 128, SP))
    xload = {}
    for vi, (va, vb) in enumerate((("xt1a", "xt1b"), ("xt2a", "xt2b"))):
        for hi, (o0, o1) in enumerate(halves):
            io.inc(SY.dma_start(xts[va][:, :, o0:o1], din[va][:, :, o0:o1]))
            io.inc(SY.dma_start(xts[vb][:, :, o0:o1], din[vb][:, :, o0:o1]))
            fence()
            xload[(vi, hi)] = io.n
        if len(halves) == 1:
            xload[(vi, 1)] = io.n

    # t12 row n = [s1a[n] | s1b[n] | s2a[n] | s2b[n]], 256 bf16 cols
    stc = {}
    nmm = 0
    for vi, (va, vb) in enumerate((("xt1a", "xt1b"), ("xt2a", "xt2b"))):
        for j in range(NB):
            if j == 0:
                PE.wait_ge(io.sem, xload[(vi, 0)])
            if j == HB:
                PE.wait_ge(io.sem, xload[(vi, 1)])
            p = mm_ps[nmm % 2]
            if nmm >= 2:
                PE.wait_ge(dv.sem, stc[nmm - 2])
            o = j * 128
            last = None
            for bi, (xv, w) in enumerate(((va, c_w1a), (vb, c_w1b))):
                base = bi * 64
                for cch in range(2):
                    last = PE.matmul(p[:, base:base + 64],
                                     xts[xv][:, cch, o:o + 128],
                                     w[:, cch, :],
                                     start=(cch == 0), stop=(cch == 1))
            pe.inc(last)
            DV.wait_ge(pe.sem, pe.n)
            dv.inc(DV.tensor_copy(tstage[:, j, vi * 128:vi * 128 + 128],
                                  p[:, 0:128]))
            stc[nmm] = dv.n
            nmm += 1
    pe_phaseA = pe.n
    SY.wait_ge(dv.sem, dv.n)
    io.inc(SY.dma_start(
        t12_in[:].rearrange("(t p) f -> p t f", p=128), tstage[:]))
    fence()
    GP.wait_ge(io.sem, io.n)
    ccs[0].inc(GP.collective_compute(
        "AllGather", AOP.bypass, replica_groups=[list(range(NCORES))],
        ins=[t12_in[:]], outs=[t12_full[:]]))
    # idx loads hide under the collective
    for a in (1, 2):
        io.inc(SY.dma_start(gidx_sb[a][:], din[f"gidx{a}"][:]))
        io.inc(SY.dma_start(dst_sb[a][:], din[f"dst{a}"][:]))
        io.inc(SY.dma_start(ew_sb[a][:], din[f"eww{a}"][:]))
    fence()
    idx_io = io.n

    # =========== edge pass machinery ===========
    psA.close()
    sbA.close()
    psL1 = ExitStack()
    blk_ps = [psL1.enter_context(nc.psum_tensor(f"blk_ps{i}", [128, 128], F32))
              for i in range(NPSUM)]
    tr2_ps = [psL1.enter_context(nc.psum_tensor(f"tr2_ps{i}", [128, 7, 64], F32))
              for i in range(2)]
    trb_ps = [psL1.enter_context(nc.psum_tensor(f"trb_ps{i}", [128, 896], BF16))
              for i in range(2)]
    msg = sb("msg", [128, RING * CALL_CHUNKS, 128], BF16)
    ohr = sb("ohr", [128, RING * CALL_CHUNKS, 128], BF16)
    agg1 = sb("agg1", [128, NB, 128], BF16)
    agg2 = sb("agg2", [128, NB, 128], BF16)
    tmp = sb("tmp", [128, NB, 128], BF16)
    tmp2 = sb("tmp2", [128, NB, 128], BF16)
    # L2 outputs alias agg2 (free after the gated combine); scratch aliases agg1
    prop1 = lambda b=None: agg2[:, :, 0:64] if b is None else agg2[:, b, 0:64]
    prop2 = lambda b=None: agg2[:, :, 64:128] if b is None else agg2[:, b, 64:128]
    t64f = lambda b=None: agg1[:, :, 0:64] if b is None else agg1[:, b, 0:64]

    scheds = {a: _sched_chunks(adjmeta[a], NB) for a in (1, 2)}
    gcall = [0]
    pe_cons_vals = []
    npass = [0]
    psum_last = {}   # psum key -> (eng, val) of evac freeing it

    # gather views
    v12 = t12_full[:].rearrange("(a b) f -> a (b f)", b=2)      # [NPAIR, 512]
    t3flat = t3_full[:].rearrange("a f -> (a f)")
    l2v = {0: t3_full[0:NPAIR, :],
           1: t3flat[64:64 + NPAIR * 128].rearrange("(a f) -> a f", f=128)}

    def edge_pass(adj, layer, dests, fold_eng_gp=False):
        """dests = (dest(b), scr(b), dest_full, scr_full)."""
        fold_eng = GP if fold_eng_gp else DV
        meta = adjmeta[adj]
        sched = scheds[adj]
        cpb0, cpb1, chp0, chp1, ns = meta
        if layer == 1:
            inap = {p: v12[:, (adj - 1) * 128 + p * 256:
                           (adj - 1) * 128 + p * 256 + 128] for p in (0, 1)}
            step = 512
        else:
            inap = l2v
            step = 128
        F = 128 if layer == 1 else 64
        dest, scr, dest_full, scr_full = dests
        first = npass[0] == 0
        npass[0] += 1
        cc_need = 0 if layer == 1 else 1
        GP.wait_ge(ccs[cc_need].sem, 1)
        if first:
            # msg/ohr alias freed phase-A space; DVE onehots must not race
            # phase-A PE reads (GP path is transitively safe via cc wait)
            DV.wait_ge(pe.sem, pe_phaseA)
        for p, chp in ((0, chp0), (1, chp1)):
            ch0 = 0 if p == 0 else chp0
            for j in range(chp // CALL_CHUNKS):
                rj = (gcall[0] % RING) * CALL_CHUNKS
                gslot = gcall[0] % RING
                cbase = ch0 + j * CALL_CHUNKS
                if gcall[0] == 0:
                    GP.wait_ge(io.sem, idx_io)
                if len(pe_cons_vals) >= RING:
                    GP.wait_ge(pe.sem, pe_cons_vals[-RING])
                g = GP.dma_gather(
                    msg[:, rj:rj + CALL_CHUNKS, :], inap[p],
                    gidx_sb[adj][:, cbase * 8:(cbase + CALL_CHUNKS) * 8],
                    S_CALL, S_CALL, 128, elem_step=step,
                    queue_num=gcall[0] % 2)
                gsems[gslot].inc(g)
                gv = gsems[gslot].n
                # onehot build
                if gcall[0] == 0:
                    DV.wait_ge(io.sem, idx_io)
                if len(pe_cons_vals) >= RING:
                    DV.wait_ge(pe.sem, pe_cons_vals[-RING])
                for c8 in range(CALL_CHUNKS):
                    ts = DV.tensor_scalar(
                        ohr[:, rj + c8, :], c_iota[:],
                        dst_sb[adj][:, cbase + c8:cbase + c8 + 1],
                        ew_sb[adj][:, cbase + c8:cbase + c8 + 1],
                        op0=AOP.is_equal, op1=AOP.mult)
                dv.inc(ts)
                ohv = dv.n
                # matmuls
                PE.wait_ge(gsems[gslot].sem, gv)
                PE.wait_ge(dv.sem, ohv)
                for c8 in range(CALL_CHUNKS):
                    pp, b, st, sp = sched[cbase + c8]
                    key = b % NPSUM
                    ptile = blk_ps[key][:, 0:F]
                    if st and key in psum_last:
                        eng, val = psum_last[key]
                        PE.wait_ge({"dv": dv.sem, "ac": ac.sem}[eng], val)
                    mmi = PE.matmul(ptile, ohr[:, rj + c8, :],
                                    msg[:, rj + c8, 0:F],
                                    start=st, stop=sp)
                    if sp:
                        pe.inc(mmi)
                        AC.wait_ge(pe.sem, pe.n)
                        cpi = AC.activation(dest(b) if pp == 0 else scr(b),
                                            ptile, ACT.Copy)
                        ac.inc(cpi)
                        psum_last[key] = ("ac", ac.n)
                if not sp:
                    pe.inc(mmi)
                pe_cons_vals.append(pe.n)
                gcall[0] += 1
        # fold the parity-1 partials into dest (one wide bf16 add)
        if fold_eng is DV:
            DV.wait_ge(ac.sem, ac.n)
            dv.inc(DV.tensor_tensor(dest_full, dest_full, scr_full, op=AOP.add))
        else:
            GP.wait_ge(ac.sem, ac.n)
            GP.tensor_tensor(dest_full, dest_full, scr_full, op=AOP.add)
            GP.drain()

    edge_pass(1, 1, (lambda b: agg1[:, b, :], lambda b: tmp[:, b, :],
                     agg1[:], tmp[:]))
    edge_pass(2, 1, (lambda b: agg2[:, b, :], lambda b: tmp2[:, b, :],
                     agg2[:], tmp2[:]), fold_eng_gp=True)

    # =========== Phase C: mid gating + L2 table ===========
    # branch 1 on DVE, branch 2 on GPSIMD (Pool idle here), all bf16
    b1b = c_b1r[:, None, :].broadcast_to([128, NB, 128])
    g1b_b = c_g1w[:, None, :].broadcast_to([128, NB, 128])
    g2b_b = c_g2w[:, None, :].broadcast_to([128, NB, 128])
    DV.drain()
    DV.tensor_tensor(agg1[:], agg1[:], b1b, op=AOP.add)
    DV.drain()
    DV.tensor_scalar(agg1[:], agg1[:], 0.0, None, op0=AOP.max)
    DV.drain()
    DV.tensor_tensor(tmp[:], agg1[:], g1b_b, op=AOP.mult)
    DV.drain()
    dv.inc(DV.tensor_reduce(lamv["l1"][:], tmp[:], axis=mybir.AxisListType.X,
                            op=AOP.add))
    l1_dv = dv.n
    GP.tensor_tensor(agg2[:], agg2[:], b1b, op=AOP.add)
    GP.drain()
    GP.tensor_scalar(agg2[:], agg2[:], 0.0, None, op0=AOP.max)
    GP.drain()
    gp.inc(GP.tensor_tensor(tmp2[:], agg2[:], g2b_b, op=AOP.mult))
    DV.wait_ge(gp.sem, gp.n)
    dv.inc(DV.tensor_reduce(lamv["l2"][:], tmp2[:], axis=mybir.AxisListType.X,
                            op=AOP.add))
    l2_dv = dv.n
    AC.wait_ge(dv.sem, l2_dv)
    AC.activation(lamv["l1"][:], lamv["l1"][:], ACT.Sigmoid, bias=cbias[:, 0:1])
    ac.inc(AC.activation(lamv["l2"][:], lamv["l2"][:], ACT.Sigmoid, bias=cbias[:, 1:2]))
    DV.wait_ge(ac.sem, ac.n)
    DV.tensor_tensor(lamv["lsum"][:], lamv["l1"][:], lamv["l2"][:], op=AOP.add)
    DV.drain()
    DV.tensor_scalar(lamv["lsum"][:], lamv["lsum"][:], 1e-12, None, op0=AOP.max)
    DV.drain()
    DV.reciprocal(lamv["lsum"][:], lamv["lsum"][:])
    DV.drain()
    DV.tensor_tensor(lamv["w0"][:], lamv["l1"][:], lamv["lsum"][:], op=AOP.mult)
    DV.tensor_tensor(lamv["w1"][:], lamv["l2"][:], lamv["lsum"][:], op=AOP.mult)
    DV.drain()
    DV.tensor_copy(wbf["w0"][:], lamv["w0"][:])
    dv.inc(DV.tensor_copy(wbf["w1"][:], lamv["w1"][:]))
    wb_dv = dv.n
    w0b = wbf["w0"][:, :, None].broadcast_to([128, NB, 128])
    w1b_ = wbf["w1"][:, :, None].broadcast_to([128, NB, 128])
    DV.drain()
    DV.tensor_tensor(agg1[:], agg1[:], w0b, op=AOP.mult)
    GP.wait_ge(dv.sem, wb_dv)
    gp.inc(GP.tensor_tensor(tmp2[:], agg2[:], w1b_, op=AOP.mult))
    DV.drain()
    DV.wait_ge(gp.sem, gp.n)
    dv.inc(DV.tensor_tensor(agg1[:], agg1[:], tmp2[:], op=AOP.add))  # x -> agg1
    xfin = dv.n


    def _s2_mm(g):
        b0 = g * GB
        nb_g = min(GB, NB - b0)
        pf = tr2_ps[g % 2]
        PE.wait_ge(dv.sem, xttc[g])
        if g >= 2:
            PE.wait_ge(dv.sem, stc2[g - 2])   # tstage copy freeing pf
        for i in range(nb_g):
            pe.inc(PE.matmul(pf[:, i, :], xttg[g % 2][:, i * 128:(i + 1) * 128],
                             c_w2[:], start=True, stop=True))
        DV.wait_ge(pe.sem, pe.n)
        dv.inc(DV.tensor_copy(tstage[:, b0:b0 + nb_g, 0:64], pf[:, 0:nb_g, :]))
        stc2[g] = dv.n
    # L2 table: s2 = x @ W2, batched 4 blocks per psum tile so the
    # transpose->copy->matmul chain amortizes sem latency over 4 blocks
    GB = 7
    ngrp = -(-NB // GB)
    stc2 = {}
    xttc = {}
    for g in range(ngrp):
        b0 = g * GB
        nb_g = min(GB, NB - b0)
        pb = trb_ps[g % 2]
        if g == 0:
            PE.wait_ge(dv.sem, xfin)
        if g >= 2:
            PE.wait_ge(dv.sem, xttc[g - 2])   # xttg slot free (copied out)
        for i in range(nb_g):
            pe.inc(PE.transpose(pb[:, i * 128:(i + 1) * 128],
                                agg1[:, b0 + i, :], c_idf[:]))
        DV.wait_ge(pe.sem, pe.n)
        dv.inc(DV.tensor_copy(xttg[g % 2][:, 0:nb_g * 128], pb[:, 0:nb_g * 128]))
        xttc[g] = dv.n
        if g >= 1:
            _s2_mm(g - 1)
    _s2_mm(ngrp - 1)
    SY.wait_ge(dv.sem, dv.n)
    io.inc(SY.dma_start(t3_in[:].rearrange("(t p) f -> p t f", p=128),
                        tstage[:, :, 0:64]))
    fence()
    GP.wait_ge(io.sem, io.n)
    ccs[1].inc(GP.collective_compute(
        "AllGather", AOP.bypass, replica_groups=[list(range(NCORES))],
        ins=[t3_in[:]],
        outs=[t3_full[0:NPAIR, :].rearrange("a (b f) -> (a b) f", b=2)]))

    # =========== L2 edge passes ===========
    edge_pass(1, 2, (lambda b: prop1(b), lambda b: tmp[:, b, 0:64],
                     prop1(), tmp[:, :, 0:64]))
    edge_pass(2, 2, (lambda b: prop2(b), lambda b: tmp2[:, b, 0:64],
                     prop2(), tmp2[:, :, 0:64]), fold_eng_gp=True)
    psL1.close()

    # =========== Phase F: final gating + outputs ===========
    b2b = c_b2r[:, None, 0:64].broadcast_to([128, NB, 64])
    h1b_b = c_h1w[:, None, :].broadcast_to([128, NB, 64])
    h2b_b = c_h2w[:, None, :].broadcast_to([128, NB, 64])
    DV.drain()
    DV.tensor_tensor(prop1(), prop1(), b2b, op=AOP.add)
    DV.drain()
    dv.inc(DV.tensor_tensor(t64f(), prop1(), h1b_b, op=AOP.mult))
    DV.drain()
    dv.inc(DV.tensor_reduce(lamv["l1"][:], t64f(), axis=mybir.AxisListType.X,
                            op=AOP.add))
    l1_dv = dv.n
    GP.tensor_tensor(prop2(), prop2(), b2b, op=AOP.add)
    GP.drain()
    gp.inc(GP.tensor_tensor(tmp2[:, :, 0:64], prop2(), h2b_b, op=AOP.mult))
    DV.wait_ge(gp.sem, gp.n)
    dv.inc(DV.tensor_reduce(lamv["l2"][:], tmp2[:, :, 0:64],
                            axis=mybir.AxisListType.X, op=AOP.add))
    l2_dv = dv.n
    SY.wait_ge(dv.sem, l2_dv)
    SY.wait_ge(gp.sem, gp.n)
    io.inc(SY.dma_start(p1_o[:].rearrange("(t p) f -> p t f", p=128),
                        agg2[:, :, 0:NCLASS]))
    io.inc(SY.dma_start(p2_o[:].rearrange("(t p) f -> p t f", p=128),
                        agg2[:, :, 64:64 + NCLASS]))
    fence()
    pout_io = io.n
    AC.wait_ge(dv.sem, l2_dv)
    AC.activation(lamv["l1"][:], lamv["l1"][:], ACT.Sigmoid, bias=cbias[:, 2:3])
    ac.inc(AC.activation(lamv["l2"][:], lamv["l2"][:], ACT.Sigmoid, bias=cbias[:, 3:4]))
    DV.wait_ge(ac.sem, ac.n)
    DV.tensor_tensor(lamv["lsum"][:], lamv["l1"][:], lamv["l2"][:], op=AOP.add)
    DV.drain()
    DV.tensor_scalar(lamv["lsum"][:], lamv["lsum"][:], 1e-12, None, op0=AOP.max)
    DV.drain()
    DV.reciprocal(lamv["lsum"][:], lamv["lsum"][:])
    DV.drain()
    DV.tensor_tensor(lamv["w0"][:], lamv["l1"][:], lamv["lsum"][:], op=AOP.mult)
    DV.tensor_tensor(lamv["w1"][:], lamv["l2"][:], lamv["lsum"][:], op=AOP.mult)
    DV.drain()
    DV.tensor_copy(wbf["w0"][:], lamv["w0"][:])
    dv.inc(DV.tensor_copy(wbf["w1"][:], lamv["w1"][:]))
    wb_dv = dv.n
    w0b6 = wbf["w0"][:, :, None].broadcast_to([128, NB, 64])
    w1b6 = wbf["w1"][:, :, None].broadcast_to([128, NB, 64])
    DV.drain()
    DV.wait_ge(io.sem, pout_io)  # don't clobber props mid-DMA
    DV.tensor_tensor(t64f(), prop1(), w0b6, op=AOP.mult)
    GP.wait_ge(dv.sem, wb_dv)
    GP.wait_ge(io.sem, pout_io)
    gp.inc(GP.tensor_tensor(tmp2[:, :, 0:64], prop2(), w1b6, op=AOP.mult))
    DV.drain()
    DV.wait_ge(gp.sem, gp.n)
    dv.inc(DV.tensor_tensor(t64f(), t64f(), tmp2[:, :, 0:64], op=AOP.add))
    SY.wait_ge(dv.sem, dv.n)
    io.inc(SY.dma_start(out_o[:].rearrange("(t p) f -> p t f", p=128),
                        agg1[:, :, 0:NCLASS]))
    SY.wait_ge(io.sem, io.n)

    nc.compile()
    ctx.close()
    return nc


def _run(inputs, sim=False):
    S = inputs["x1a"].shape[0] // NCORES
    NB = -(-S // 128)
    SP = NB * 128
    NROWS = NCORES * SP

    adj = {}
    adjmeta = {}
    for a in (1, 2):
        out, meta = _prep_adjacency(
            inputs[f"src{a}"], inputs[f"dst{a}"], inputs[f"ew{a}"],
            S, SP, NB, NROWS)
        adj[a] = out
        adjmeta[a] = meta

    scalars = (float(np.asarray(inputs["g1b"]).ravel()[0]),
               float(np.asarray(inputs["g2b"]).ravel()[0]),
               float(np.asarray(inputs["h1b"]).ravel()[0]),
               float(np.asarray(inputs["h2b"]).ravel()[0]))
    nc = _build(S, SP, NB, NROWS, adjmeta, scalars)

    bf = ml_dtypes.bfloat16
    f32 = np.float32

    def wfmt(w):  # [256, 64] -> [128, 2, 64] bf16
        return np.ascontiguousarray(
            np.asarray(w, f32).reshape(2, 128, NHID).transpose(1, 0, 2)).astype(bf)

    w2pad = np.zeros((128, 64), f32)
    w2pad[:, :NCLASS] = np.asarray(inputs["W2"], f32)
    iota = np.tile(np.arange(128, dtype=f32), (128, 1))
    ident = np.eye(128, dtype=f32)
    g1w = np.tile(np.asarray(inputs["g1w"], f32).ravel(), (128, 1))
    g2w = np.tile(np.asarray(inputs["g2w"], f32).ravel(), (128, 1))
    h1w = np.zeros((128, 64), f32)
    h1w[:, :NCLASS] = np.asarray(inputs["h1w"], f32).ravel()
    h2w = np.zeros((128, 64), f32)
    h2w[:, :NCLASS] = np.asarray(inputs["h2w"], f32).ravel()
    b1r = np.tile(np.concatenate([np.asarray(inputs["b1a"], f32).ravel(),
                                  np.asarray(inputs["b1b"], f32).ravel()]), (128, 1))
    b2r = np.zeros((128, 64), f32)
    b2r[:, :NCLASS] = np.asarray(inputs["b2"], f32).ravel()

    common = dict(
        w1a=wfmt(inputs["W1a"]), w1b=wfmt(inputs["W1b"]),
        w2=w2pad.astype(bf), iota=iota.astype(bf), idf=ident.astype(bf),
        g1w=g1w.astype(bf), g2w=g2w.astype(bf),
        h1w=h1w.astype(bf), h2w=h2w.astype(bf),
        b1r=b1r.astype(bf), b2r=b2r.astype(bf))

    def xfmt(x, k):  # shard k, pad, transpose -> [128, 2, SP] bf16
        xs = np.asarray(x, f32)[k * S:(k + 1) * S]
        xp = np.zeros((SP, NFEAT), f32)
        xp[:S] = xs
        xt = xp.T.reshape(2, 128, SP).transpose(1, 0, 2)
        return np.ascontiguousarray(xt).astype(bf)

    in_maps = []
    for k in range(NCORES):
        m = dict(common)
        for v, key in (("xt1a", "x1a"), ("xt1b", "x1b"),
                       ("xt2a", "x2a"), ("xt2b", "x2b")):
            m[v] = xfmt(inputs[key], k)
        for a in (1, 2):
            g, d, e = adj[a][k]
            m[f"gidx{a}"] = g
            m[f"dst{a}"] = d
            m[f"eww{a}"] = e
        in_maps.append(m)

    global LAST_EXEC_NS
    if sim:
        from concourse.bass_interp import MultiCoreSim
        msim = MultiCoreSim(nc, NCORES)
        for k in range(NCORES):
            for name, arr in in_maps[k].items():
                msim.cores[k].tensor(name)[:] = arr
        msim.simulate()
        results = [{nm: msim.cores[k].tensor(nm).copy()
                    for nm in ("out_o", "p1_o", "p2_o")} for k in range(NCORES)]
    else:
        import os
        r = run_bass_kernel_spmd(nc, in_maps, list(range(NCORES)))
        LAST_EXEC_NS = r.exec_time_ns
        results = r.results

    outs = []
    for nm in ("out_o", "p1_o", "p2_o"):
        outs.append(np.concatenate([results[k][nm][:S] for k in range(NCORES)],
                    axis=0).astype(np.float32))
    return tuple(outs)


LAST_EXEC_NS = None


def kernel(**inputs):
    return _run(inputs, sim=False)


# revision 5
# speedup vs baseline: 1.0337x; 1.0121x over previous
"""DaGCN on 8 Trainium2 NeuronCores (Bass SPMD) — v2.

Changes vs v1 (1472us cost-model):
  * ONE merged AllGather for the L1 tables: t12 = [s1|s2] rows [SP, 256]
    bf16 -> out 25.7MB @ ~285us (vs 2x252us), exploiting the collective
    cost model's concave bandwidth ramp.
  * L2 table collective gathers the PACKED [SP, 64] bf16 shard (out 6.4MB
    @ ~176us vs 252us).  Gathers index NODE PAIRS (row//2, int16-safe for
    all 50176 rows) and select the even/odd node via the 256B-aligned
    gather offset; edges are split by src-row parity instead of lo/hi
    halves, so L1 and L2 share one set of idx/dst/ew arrays, loaded once.
  * Phase A computes the s-tables node-major directly (no transposes),
    x loads split in halves so PE starts early; idx loads hide under the
    first collective.
  * PSUM evacuations go to ACT (both parity runs), folded with one wide
    bf16 DVE add per pass; gating math runs in bf16 split DVE/GPSIMD.
  * The L2-table transpose+matmul chain is batched 7 dst-blocks per psum
    tile so cross-engine semaphore latency amortizes (30us -> ~10us); the
    phase-A matmul loop likewise batches 4 node-slices per psum tile with
    evac copies alternating DVE/ACT, runs halves-major so each t12
    row-half DMAs while the other half computes, and defers all non-W
    constant loads under the first collective.
  * Edge meta (gidx/dst/ew) stays SBUF-resident for all 4 passes.
"""

import math
from contextlib import ExitStack

import ml_dtypes
import numpy as np

import concourse.bacc as bacc
import concourse.bass as bass
import concourse.mybir as mybir
from concourse.bass_utils import run_bass_kernel_spmd

F32 = mybir.dt.float32
BF16 = mybir.dt.bfloat16
I16 = mybir.dt.int16
AOP = mybir.AluOpType
ACT = mybir.ActivationFunctionType

NCORES = 8
N = 50000
NFEAT, NHID, NCLASS = 256, 64, 32
S_CALL = 1024          # idxs per dma_gather call (HW-validated; 2048 hangs)
CALL_CHUNKS = S_CALL // 128
RING = 16              # gather/onehot ring depth (in calls)
NPSUM = 4              # psum block-accumulator ring (L1)


def _wrap16(a):
    """[n] int16 -> [128, n//16]: idx i at [i%16, i//16], replicated x8."""
    n = a.shape[0]
    w = a.reshape(n // 16, 16).T.astype(np.int16)
    return np.tile(w, (8, 1)).copy()


def _chunkwrap(a, dtype):
    """[n] -> [128, n//128]: edge i at [i%128, i//128]."""
    n = a.shape[0]
    return np.ascontiguousarray(a.reshape(n // 128, 128).T.astype(dtype))


def _prep_adjacency(src, dst, ew, S, SP, NB, NROWS):
    """Bucket edges by (dst core, src-row parity, dst block).

    Returns per-core (gidx, dcol, eww) arrays plus the shared compile-time
    schedule: cpb[p][b] = chunks for (parity p, block b), identical across
    cores (max), with per-parity chunk counts padded to CALL_CHUNKS.
    """
    src = np.asarray(src).astype(np.int64)
    dst = np.asarray(dst).astype(np.int64)
    ew = np.asarray(ew).astype(np.float32)
    core = dst // S
    row = (src // S) * SP + (src % S)       # padded table row
    par = row % 2
    pair = row // 2
    dstrel = dst - core * S
    blk = dstrel // 128
    col = dstrel % 128

    percore = []
    counts = np.zeros((NCORES, 2, NB), np.int64)
    for k in range(NCORES):
        m = core == k
        e = np.lexsort((blk[m], par[m]))   # sort by (parity, block)
        r, h, b, c, w = pair[m][e], par[m][e], blk[m][e], col[m][e], ew[m][e]
        percore.append((r, h, b, c, w))
        for p in range(2):
            mm = h == p
            counts[k, p] = np.bincount(b[mm], minlength=NB)

    cpb = np.maximum(np.ceil(counts.max(axis=0) / 128).astype(np.int64), 1)
    # parity-1 runs may be empty only if parity-0 handles init; keep >=1 on p0
    ch = [int(cpb[p].sum()) for p in range(2)]
    chp = [-(-c // CALL_CHUNKS) * CALL_CHUNKS for c in ch]
    # trailing pad chunks extend block NB-1's run of that parity
    nslot = (chp[0] + chp[1]) * 128

    # chunk offset of (p, b)
    coff = np.zeros((2, NB), np.int64)
    coff[0] = np.concatenate(([0], np.cumsum(cpb[0])))[:-1]
    coff[1] = chp[0] + np.concatenate(([0], np.cumsum(cpb[1])))[:-1]

    out = []
    for k in range(NCORES):
        r, h, b, c, w = percore[k]
        gidx = np.zeros(nslot, np.int64)
        dcol = np.zeros(nslot, np.int64)
        eww = np.zeros(nslot, np.float32)
        for p in range(2):
            mm = h == p
            rr, bb, cc, ww = r[mm], b[mm], c[mm], w[mm]
            cnt = counts[k, p]
            offs = np.concatenate(([0], np.cumsum(cnt)))[:-1]
            pos = np.arange(rr.shape[0]) - offs[bb]
            slot = (coff[p][bb]) * 128 + pos
            gidx[slot] = rr
            dcol[slot] = cc
            eww[slot] = ww
        out.append((
            _wrap16(gidx),
            _chunkwrap(dcol, np.float32),
            _chunkwrap(eww, np.float32),
        ))
    return out, (cpb[0].tolist(), cpb[1].tolist(), chp[0], chp[1], nslot)


def _sched_chunks(meta, NB):
    """Per global chunk: (parity, block, start, stop). Pads extend the last
    block's run of their parity."""
    cpb0, cpb1, chp0, chp1, _ = meta
    sched = []
    for p, cpbs, chp in ((0, cpb0, chp0), (1, cpb1, chp1)):
        for b in range(NB):
            for i in range(cpbs[b]):
                sched.append([p, b, i == 0, False])
        for _ in range(chp - sum(cpbs)):
            sched.append([p, NB - 1, False, False])
    # mark stops: last chunk of each (p, b)
    last = {}
    for i, (p, b, st, sp) in enumerate(sched):
        last[(p, b)] = i
    for (p, b), i in last.items():
        sched[i][3] = True
    return sched


class Ctr:
    def __init__(self, sem, step=1):
        self.sem, self.n, self.step = sem, 0, step

    def inc(self, inst):
        inst.then_inc(self.sem, self.step)
        self.n += self.step
        return self.n


def _build(S, SP, NB, NROWS, adjmeta, scalars):
    nc = bacc.Bacc("TRN2", num_devices=NCORES, num_swdge_queues=2)
    g1b, g2b, h1b, h2b = scalars
    NPAIR = NROWS // 2

    # ---------------- I/O ----------------
    din = {}
    for v in ("xt1a", "xt1b", "xt2a", "xt2b"):
        din[v] = nc.dram_tensor(v, [128, 2, SP], BF16, kind="ExternalInput")
    din["w1a"] = nc.dram_tensor("w1a", [128, 2, NHID], BF16, kind="ExternalInput")
    din["w1b"] = nc.dram_tensor("w1b", [128, 2, NHID], BF16, kind="ExternalInput")
    din["w2"] = nc.dram_tensor("w2", [128, 64], BF16, kind="ExternalInput")
    din["iota"] = nc.dram_tensor("iota", [128, 128], BF16, kind="ExternalInput")
    din["idf"] = nc.dram_tensor("idf", [128, 128], BF16, kind="ExternalInput")
    din["g1w"] = nc.dram_tensor("g1w", [128, 128], BF16, kind="ExternalInput")
    din["g2w"] = nc.dram_tensor("g2w", [128, 128], BF16, kind="ExternalInput")
    din["h1w"] = nc.dram_tensor("h1w", [128, 64], BF16, kind="ExternalInput")
    din["h2w"] = nc.dram_tensor("h2w", [128, 64], BF16, kind="ExternalInput")
    din["b1r"] = nc.dram_tensor("b1r", [128, 128], BF16, kind="ExternalInput")
    din["b2r"] = nc.dram_tensor("b2r", [128, 64], BF16, kind="ExternalInput")
    for a in (1, 2):
        ns = adjmeta[a][4]
        din[f"gidx{a}"] = nc.dram_tensor(f"gidx{a}", [128, ns // 16], I16, kind="ExternalInput")
        din[f"dst{a}"] = nc.dram_tensor(f"dst{a}", [128, ns // 128], F32, kind="ExternalInput")
        din[f"eww{a}"] = nc.dram_tensor(f"eww{a}", [128, ns // 128], F32, kind="ExternalInput")
    out_o = nc.dram_tensor("out_o", [SP, NCLASS], BF16, kind="ExternalOutput")
    p1_o = nc.dram_tensor("p1_o", [SP, NCLASS], BF16, kind="ExternalOutput")
    p2_o = nc.dram_tensor("p2_o", [SP, NCLASS], BF16, kind="ExternalOutput")

    t12_in = nc.dram_tensor("t12in", [SP, 256], BF16)
    t12_full = nc.dram_tensor("t12full", [NROWS, 256], BF16, addr_space="Shared")
    t3_in = nc.dram_tensor("t3in", [SP, NHID], BF16)
    # one pad row so the odd-parity strided gather view stays in bounds
    t3_full = nc.dram_tensor("t3full", [NPAIR + 1, 128], BF16, addr_space="Shared")

    ctx = ExitStack()
    sb = lambda name, shape, dt: ctx.enter_context(nc.sbuf_tensor(name, shape, dt))
    ps = lambda name, shape: ctx.enter_context(nc.psum_tensor(name, shape, F32))
    sem = lambda name: ctx.enter_context(nc.semaphore(name))

    # ---------------- SBUF ----------------
    c_w1a = sb("c_w1a", [128, 2, NHID], BF16)
    c_w1b = sb("c_w1b", [128, 2, NHID], BF16)
    c_w2 = sb("c_w2", [128, 64], BF16)
    c_iota = sb("c_iota", [128, 128], BF16)
    c_idf = sb("c_idf", [128, 128], BF16)
    c_g1w = sb("c_g1w", [128, 128], BF16)
    c_g2w = sb("c_g2w", [128, 128], BF16)
    c_h1w = sb("c_h1w", [128, 64], BF16)
    c_h2w = sb("c_h2w", [128, 64], BF16)
    c_b1r = sb("c_b1r", [128, 128], BF16)
    c_b2r = sb("c_b2r", [128, 64], BF16)
    cbias = sb("cbias", [128, 4], F32)

    tstage = sb("tstage", [128, NB, 256], BF16)
    ns1 = adjmeta[1][4]
    ns2 = adjmeta[2][4]
    gidx_sb = {1: sb("gidx1_sb", [128, ns1 // 16], I16),
               2: sb("gidx2_sb", [128, ns2 // 16], I16)}
    dst_sb = {1: sb("dst1_sb", [128, ns1 // 128], F32),
              2: sb("dst2_sb", [128, ns2 // 128], F32)}
    ew_sb = {1: sb("ew1_sb", [128, ns1 // 128], F32),
             2: sb("ew2_sb", [128, ns2 // 128], F32)}
    lamv = {nm: sb(nm, [128, NB], F32)
            for nm in ("l1", "l2", "lsum", "w0", "w1")}
    wbf = {nm: sb(f"wb_{nm}", [128, NB], BF16) for nm in ("w0", "w1")}
    xtt = sb("xtt", [128, 128], BF16)
    xttg = [sb(f"xttg{i}", [128, 896], BF16) for i in range(2)]

    sbA = ExitStack()
    xts = {v: sbA.enter_context(nc.sbuf_tensor(f"x{v}", [128, 2, SP], BF16))
           for v in ("xt1a", "xt1b", "xt2a", "xt2b")}

    psA = ExitStack()
    mm_ps = [psA.enter_context(nc.psum_tensor(f"mm_ps{i}", [128, 4, 128], F32))
             for i in range(4)]

    io = Ctr(sem("io"), 16)        # sync-engine DMAs
    gsems = [Ctr(sem(f"g{i}"), 16) for i in range(RING)]
    ccs = [Ctr(sem(f"cc{i}"), 1) for i in range(2)]
    pe = Ctr(sem("pe"), 1)
    dv = Ctr(sem("dv"), 1)
    ac = Ctr(sem("ac"), 1)
    gp = Ctr(sem("gp"), 1)

    SY, PE, DV, AC, GP = nc.sync, nc.tensor, nc.vector, nc.scalar, nc.gpsimd

    def fence():
        SY.wait_ge(io.sem, io.n)

    # =========== Phase A: constants + node-major s tables ===========
    for bi, bval in enumerate((g1b, g2b, h1b, h2b)):
        nc.vector.memset(cbias[:, bi:bi + 1], float(bval))
    dv.inc(DV.memset(xtt[:], 0))
    for name, t in (("w1a", c_w1a), ("w1b", c_w1b)):
        io.inc(SY.dma_start(t[:], din[name][:]))
    HB = NB // 2 + 1          # node-slice halves for load/compute overlap
    halves = [(0, min(HB * 128, SP))]
    if HB * 128 < SP:
        halves.append((HB * 128, SP))
    xload = {}
    for hi, (o0, o1) in enumerate(halves):
        for vi, (va, vb) in enumerate((("xt1a", "xt1b"), ("xt2a", "xt2b"))):
            io.inc(SY.dma_start(xts[va][:, :, o0:o1], din[va][:, :, o0:o1]))
            io.inc(SY.dma_start(xts[vb][:, :, o0:o1], din[vb][:, :, o0:o1]))
            fence()
            xload[(vi, hi)] = io.n
    if len(halves) == 1:
        for vi in range(2):
            xload[(vi, 1)] = xload[(vi, 0)]

    # t12 row n = [s1a[n] | s1b[n] | s2a[n] | s2b[n]], 256 bf16 cols.
    # Batch 4 node-slices per psum tile (4-deep ring) with wide evac copies
    # alternating DVE/ACT so the copy sem round-trip amortizes over 4 slices.
    GA = 4
    h0_blocks = min(HB, NB)
    segs = [(0, h0_blocks)]
    if h0_blocks < NB:
        segs.append((h0_blocks, NB))
    stc = {}
    gctr = 0
    for hi, (b0, b1) in enumerate(segs):
        for vi, (va, vb) in enumerate((("xt1a", "xt1b"), ("xt2a", "xt2b"))):
            PE.wait_ge(io.sem, xload[(vi, hi)])
            j0 = b0
            while j0 < b1:
                nj = min(GA, b1 - j0)
                p = mm_ps[gctr % 4]
                if gctr >= 4:
                    eng, val = stc[gctr - 4]
                    PE.wait_ge(dv.sem if eng == "dv" else ac.sem, val)
                last = None
                for i in range(nj):
                    o = (j0 + i) * 128
                    for bi, (xv, w) in enumerate(((va, c_w1a), (vb, c_w1b))):
                        base = bi * 64
                        for cch in range(2):
                            last = PE.matmul(p[:, i, base:base + 64],
                                             xts[xv][:, cch, o:o + 128],
                                             w[:, cch, :],
                                             start=(cch == 0), stop=(cch == 1))
                pe.inc(last)
                dst = tstage[:, j0:j0 + nj, vi * 128:vi * 128 + 128]
                if gctr % 2 == 0:
                    DV.wait_ge(pe.sem, pe.n)
                    dv.inc(DV.tensor_copy(dst, p[:, 0:nj, :]))
                    stc[gctr] = ("dv", dv.n)
                else:
                    AC.wait_ge(pe.sem, pe.n)
                    ac.inc(AC.activation(dst, p[:, 0:nj, :], ACT.Copy))
                    stc[gctr] = ("ac", ac.n)
                gctr += 1
                j0 += nj
        # both views done for this node range: ship its t12 rows now
        SY.wait_ge(dv.sem, dv.n)
        SY.wait_ge(ac.sem, ac.n)
        io.inc(SY.dma_start(
            t12_in[b0 * 128:b1 * 128, :].rearrange("(t p) f -> p t f", p=128),
            tstage[:, b0:b1, :]))
        fence()
    t12_io = io.n
    pe_phaseA = pe.n
    GP.wait_ge(io.sem, t12_io)
    ccs[0].inc(GP.collective_compute(
        "AllGather", AOP.bypass, replica_groups=[list(range(NCORES))],
        ins=[t12_in[:]], outs=[t12_full[:]]))
    # consts, the t3 pad row, and idx loads all hide under the collective
    SY.wait_ge(dv.sem, 1)   # xtt memset (first dv op)
    io.inc(SY.dma_start(t3_full[NPAIR:NPAIR + 1, :], xtt[0:1, :]))
    for name, t in (("w2", c_w2), ("iota", c_iota), ("idf", c_idf),
                    ("g1w", c_g1w), ("g2w", c_g2w), ("h1w", c_h1w),
                    ("h2w", c_h2w), ("b1r", c_b1r), ("b2r", c_b2r)):
        io.inc(SY.dma_start(t[:], din[name][:]))
    for a in (1, 2):
        io.inc(SY.dma_start(gidx_sb[a][:], din[f"gidx{a}"][:]))
        io.inc(SY.dma_start(dst_sb[a][:], din[f"dst{a}"][:]))
        io.inc(SY.dma_start(ew_sb[a][:], din[f"eww{a}"][:]))
    fence()
    idx_io = io.n

    # =========== edge pass machinery ===========
    psA.close()
    sbA.close()
    psL1 = ExitStack()
    blk_ps = [psL1.enter_context(nc.psum_tensor(f"blk_ps{i}", [128, 128], F32))
              for i in range(NPSUM)]
    tr2_ps = [psL1.enter_context(nc.psum_tensor(f"tr2_ps{i}", [128, 7, 64], F32))
              for i in range(2)]
    trb_ps = [psL1.enter_context(nc.psum_tensor(f"trb_ps{i}", [128, 896], BF16))
              for i in range(2)]
    msg = sb("msg", [128, RING * CALL_CHUNKS, 128], BF16)
    ohr = sb("ohr", [128, RING * CALL_CHUNKS, 128], BF16)
    agg1 = sb("agg1", [128, NB, 128], BF16)
    agg2 = sb("agg2", [128, NB, 128], BF16)
    tmp = sb("tmp", [128, NB, 128], BF16)
    tmp2 = sb("tmp2", [128, NB, 128], BF16)
    # L2 outputs alias agg2 (free after the gated combine); scratch aliases agg1
    prop1 = lambda b=None: agg2[:, :, 0:64] if b is None else agg2[:, b, 0:64]
    prop2 = lambda b=None: agg2[:, :, 64:128] if b is None else agg2[:, b, 64:128]
    t64f = lambda b=None: agg1[:, :, 0:64] if b is None else agg1[:, b, 0:64]

    scheds = {a: _sched_chunks(adjmeta[a], NB) for a in (1, 2)}
    gcall = [0]
    pe_cons_vals = []
    npass = [0]
    psum_last = {}   # psum key -> (eng, val) of evac freeing it

    # gather views
    v12 = t12_full[:].rearrange("(a b) f -> a (b f)", b=2)      # [NPAIR, 512]
    t3flat = t3_full[:].rearrange("a f -> (a f)")
    l2v = {0: t3_full[0:NPAIR, :],
           1: t3flat[64:64 + NPAIR * 128].rearrange("(a f) -> a f", f=128)}

    def edge_pass(adj, layer, dests, fold_eng_gp=False):
        """dests = (dest(b), scr(b), dest_full, scr_full)."""
        fold_eng = GP if fold_eng_gp else DV
        meta = adjmeta[adj]
        sched = scheds[adj]
        cpb0, cpb1, chp0, chp1, ns = meta
        if layer == 1:
            inap = {p: v12[:, (adj - 1) * 128 + p * 256:
                           (adj - 1) * 128 + p * 256 + 128] for p in (0, 1)}
            step = 512
        else:
            inap = l2v
            step = 128
        F = 128 if layer == 1 else 64
        dest, scr, dest_full, scr_full = dests
        first = npass[0] == 0
        npass[0] += 1
        cc_need = 0 if layer == 1 else 1
        GP.wait_ge(ccs[cc_need].sem, 1)
        if first:
            # msg/ohr alias freed phase-A space; DVE onehots must not race
            # phase-A PE reads (GP path is transitively safe via cc wait)
            DV.wait_ge(pe.sem, pe_phaseA)
        for p, chp in ((0, chp0), (1, chp1)):
            ch0 = 0 if p == 0 else chp0
            for j in range(chp // CALL_CHUNKS):
                rj = (gcall[0] % RING) * CALL_CHUNKS
                gslot = gcall[0] % RING
                cbase = ch0 + j * CALL_CHUNKS
                if gcall[0] == 0:
                    GP.wait_ge(io.sem, idx_io)
                if len(pe_cons_vals) >= RING:
                    GP.wait_ge(pe.sem, pe_cons_vals[-RING])
                g = GP.dma_gather(
                    msg[:, rj:rj + CALL_CHUNKS, :], inap[p],
                    gidx_sb[adj][:, cbase * 8:(cbase + CALL_CHUNKS) * 8],
                    S_CALL, S_CALL, 128, elem_step=step,
                    queue_num=gcall[0] % 2)
                gsems[gslot].inc(g)
                gv = gsems[gslot].n
                # onehot build
                if gcall[0] == 0:
                    DV.wait_ge(io.sem, idx_io)
                if len(pe_cons_vals) >= RING:
                    DV.wait_ge(pe.sem, pe_cons_vals[-RING])
                for c8 in range(CALL_CHUNKS):
                    ts = DV.tensor_scalar(
                        ohr[:, rj + c8, :], c_iota[:],
                        dst_sb[adj][:, cbase + c8:cbase + c8 + 1],
                        ew_sb[adj][:, cbase + c8:cbase + c8 + 1],
                        op0=AOP.is_equal, op1=AOP.mult)
                dv.inc(ts)
                ohv = dv.n
                # matmuls
                PE.wait_ge(gsems[gslot].sem, gv)
                PE.wait_ge(dv.sem, ohv)
                for c8 in range(CALL_CHUNKS):
                    pp, b, st, sp = sched[cbase + c8]
                    key = b % NPSUM
                    ptile = blk_ps[key][:, 0:F]
                    if st and key in psum_last:
                        eng, val = psum_last[key]
                        PE.wait_ge({"dv": dv.sem, "ac": ac.sem}[eng], val)
                    mmi = PE.matmul(ptile, ohr[:, rj + c8, :],
                                    msg[:, rj + c8, 0:F],
                                    start=st, stop=sp)
                    if sp:
                        pe.inc(mmi)
                        AC.wait_ge(pe.sem, pe.n)
                        cpi = AC.activation(dest(b) if pp == 0 else scr(b),
                                            ptile, ACT.Copy)
                        ac.inc(cpi)
                        psum_last[key] = ("ac", ac.n)
                if not sp:
                    pe.inc(mmi)
                pe_cons_vals.append(pe.n)
                gcall[0] += 1
        # fold the parity-1 partials into dest (one wide bf16 add)
        if fold_eng is DV:
            DV.wait_ge(ac.sem, ac.n)
            dv.inc(DV.tensor_tensor(dest_full, dest_full, scr_full, op=AOP.add))
        else:
            GP.wait_ge(ac.sem, ac.n)
            GP.tensor_tensor(dest_full, dest_full, scr_full, op=AOP.add)
            GP.drain()

    edge_pass(1, 1, (lambda b: agg1[:, b, :], lambda b: tmp[:, b, :],
                     agg1[:], tmp[:]))
    edge_pass(2, 1, (lambda b: agg2[:, b, :], lambda b: tmp2[:, b, :],
                     agg2[:], tmp2[:]), fold_eng_gp=True)

    # =========== Phase C: mid gating + L2 table ===========
    # branch 1 on DVE, branch 2 on GPSIMD (Pool idle here), all bf16
    b1b = c_b1r[:, None, :].broadcast_to([128, NB, 128])
    g1b_b = c_g1w[:, None, :].broadcast_to([128, NB, 128])
    g2b_b = c_g2w[:, None, :].broadcast_to([128, NB, 128])
    DV.drain()
    DV.tensor_tensor(agg1[:], agg1[:], b1b, op=AOP.add)
    DV.drain()
    DV.tensor_scalar(agg1[:], agg1[:], 0.0, None, op0=AOP.max)
    DV.drain()
    DV.tensor_tensor(tmp[:], agg1[:], g1b_b, op=AOP.mult)
    DV.drain()
    dv.inc(DV.tensor_reduce(lamv["l1"][:], tmp[:], axis=mybir.AxisListType.X,
                            op=AOP.add))
    l1_dv = dv.n
    GP.tensor_tensor(agg2[:], agg2[:], b1b, op=AOP.add)
    GP.drain()
    GP.tensor_scalar(agg2[:], agg2[:], 0.0, None, op0=AOP.max)
    GP.drain()
    gp.inc(GP.tensor_tensor(tmp2[:], agg2[:], g2b_b, op=AOP.mult))
    DV.wait_ge(gp.sem, gp.n)
    dv.inc(DV.tensor_reduce(lamv["l2"][:], tmp2[:], axis=mybir.AxisListType.X,
                            op=AOP.add))
    l2_dv = dv.n
    AC.wait_ge(dv.sem, l2_dv)
    AC.activation(lamv["l1"][:], lamv["l1"][:], ACT.Sigmoid, bias=cbias[:, 0:1])
    ac.inc(AC.activation(lamv["l2"][:], lamv["l2"][:], ACT.Sigmoid, bias=cbias[:, 1:2]))
    DV.wait_ge(ac.sem, ac.n)
    DV.tensor_tensor(lamv["lsum"][:], lamv["l1"][:], lamv["l2"][:], op=AOP.add)
    DV.drain()
    DV.tensor_scalar(lamv["lsum"][:], lamv["lsum"][:], 1e-12, None, op0=AOP.max)
    DV.drain()
    DV.reciprocal(lamv["lsum"][:], lamv["lsum"][:])
    DV.drain()
    DV.tensor_tensor(lamv["w0"][:], lamv["l1"][:], lamv["lsum"][:], op=AOP.mult)
    DV.tensor_tensor(lamv["w1"][:], lamv["l2"][:], lamv["lsum"][:], op=AOP.mult)
    DV.drain()
    DV.tensor_copy(wbf["w0"][:], lamv["w0"][:])
    dv.inc(DV.tensor_copy(wbf["w1"][:], lamv["w1"][:]))
    wb_dv = dv.n
    w0b = wbf["w0"][:, :, None].broadcast_to([128, NB, 128])
    w1b_ = wbf["w1"][:, :, None].broadcast_to([128, NB, 128])
    DV.drain()
    DV.tensor_tensor(agg1[:], agg1[:], w0b, op=AOP.mult)
    GP.wait_ge(dv.sem, wb_dv)
    gp.inc(GP.tensor_tensor(tmp2[:], agg2[:], w1b_, op=AOP.mult))
    DV.drain()
    DV.wait_ge(gp.sem, gp.n)
    dv.inc(DV.tensor_tensor(agg1[:], agg1[:], tmp2[:], op=AOP.add))  # x -> agg1
    xfin = dv.n


    def _s2_mm(g):
        b0 = g * GB
        nb_g = min(GB, NB - b0)
        pf = tr2_ps[g % 2]
        PE.wait_ge(dv.sem, xttc[g])
        if g >= 2:
            PE.wait_ge(dv.sem, stc2[g - 2])   # tstage copy freeing pf
        for i in range(nb_g):
            pe.inc(PE.matmul(pf[:, i, :], xttg[g % 2][:, i * 128:(i + 1) * 128],
                             c_w2[:], start=True, stop=True))
        DV.wait_ge(pe.sem, pe.n)
        dv.inc(DV.tensor_copy(tstage[:, b0:b0 + nb_g, 0:64], pf[:, 0:nb_g, :]))
        stc2[g] = dv.n
    # L2 table: s2 = x @ W2, batched 4 blocks per psum tile so the
    # transpose->copy->matmul chain amortizes sem latency over 4 blocks
    GB = 7
    ngrp = -(-NB // GB)
    stc2 = {}
    xttc = {}
    for g in range(ngrp):
        b0 = g * GB
        nb_g = min(GB, NB - b0)
        pb = trb_ps[g % 2]
        if g == 0:
            PE.wait_ge(dv.sem, xfin)
        if g >= 2:
            PE.wait_ge(dv.sem, xttc[g - 2])   # xttg slot free (copied out)
        for i in range(nb_g):
            pe.inc(PE.transpose(pb[:, i * 128:(i + 1) * 128],
                                agg1[:, b0 + i, :], c_idf[:]))
        DV.wait_ge(pe.sem, pe.n)
        dv.inc(DV.tensor_copy(xttg[g % 2][:, 0:nb_g * 128], pb[:, 0:nb_g * 128]))
        xttc[g] = dv.n
        if g >= 1:
            _s2_mm(g - 1)
    _s2_mm(ngrp - 1)
    SY.wait_ge(dv.sem, dv.n)
    io.inc(SY.dma_start(t3_in[:].rearrange("(t p) f -> p t f", p=128),
                        tstage[:, :, 0:64]))
    fence()
    GP.wait_ge(io.sem, io.n)
    ccs[1].inc(GP.collective_compute(
        "AllGather", AOP.bypass, replica_groups=[list(range(NCORES))],
        ins=[t3_in[:]],
        outs=[t3_full[0:NPAIR, :].rearrange("a (b f) -> (a b) f", b=2)]))

    # =========== L2 edge passes ===========
    edge_pass(1, 2, (lambda b: prop1(b), lambda b: tmp[:, b, 0:64],
                     prop1(), tmp[:, :, 0:64]))
    edge_pass(2, 2, (lambda b: prop2(b), lambda b: tmp2[:, b, 0:64],
                     prop2(), tmp2[:, :, 0:64]), fold_eng_gp=True)
    psL1.close()

    # =========== Phase F: final gating + outputs ===========
    b2b = c_b2r[:, None, 0:64].broadcast_to([128, NB, 64])
    h1b_b = c_h1w[:, None, :].broadcast_to([128, NB, 64])
    h2b_b = c_h2w[:, None, :].broadcast_to([128, NB, 64])
    DV.drain()
    DV.tensor_tensor(prop1(), prop1(), b2b, op=AOP.add)
    DV.drain()
    dv.inc(DV.tensor_tensor(t64f(), prop1(), h1b_b, op=AOP.mult))
    DV.drain()
    dv.inc(DV.tensor_reduce(lamv["l1"][:], t64f(), axis=mybir.AxisListType.X,
                            op=AOP.add))
    l1_dv = dv.n
    GP.tensor_tensor(prop2(), prop2(), b2b, op=AOP.add)
    GP.drain()
    gp.inc(GP.tensor_tensor(tmp2[:, :, 0:64], prop2(), h2b_b, op=AOP.mult))
    DV.wait_ge(gp.sem, gp.n)
    dv.inc(DV.tensor_reduce(lamv["l2"][:], tmp2[:, :, 0:64],
                            axis=mybir.AxisListType.X, op=AOP.add))
    l2_dv = dv.n
    SY.wait_ge(dv.sem, l2_dv)
    SY.wait_ge(gp.sem, gp.n)
    io.inc(SY.dma_start(p1_o[:].rearrange("(t p) f -> p t f", p=128),
                        agg2[:, :, 0:NCLASS]))
    io.inc(SY.dma_start(p2_o[:].rearrange("(t p) f -> p t f", p=128),
                        agg2[:, :, 64:64 + NCLASS]))
    fence()
    pout_io = io.n
    AC.wait_ge(dv.sem, l2_dv)
    AC.activation(lamv["l1"][:], lamv["l1"][:], ACT.Sigmoid, bias=cbias[:, 2:3])
    ac.inc(AC.activation(lamv["l2"][:], lamv["l2"][:], ACT.Sigmoid, bias=cbias[:, 3:4]))
    DV.wait_ge(ac.sem, ac.n)
    DV.tensor_tensor(lamv["lsum"][:], lamv["l1"][:], lamv["l2"][:], op=AOP.add)
    DV.drain()
    DV.tensor_scalar(lamv["lsum"][:], lamv["lsum"][:], 1e-12, None, op0=AOP.max)
    DV.drain()
    DV.reciprocal(lamv["lsum"][:], lamv["lsum"][:])
    DV.drain()
    DV.tensor_tensor(lamv["w0"][:], lamv["l1"][:], lamv["lsum"][:], op=AOP.mult)
    DV.tensor_tensor(lamv["w1"][:], lamv["l2"][:], lamv["lsum"][:], op=AOP.mult)
    DV.drain()
    DV.tensor_copy(wbf["w0"][:], lamv["w0"][:])
    dv.inc(DV.tensor_copy(wbf["w1"][:], lamv["w1"][:]))
    wb_dv = dv.n
    w0b6 = wbf["w0"][:, :, None].broadcast_to([128, NB, 64])
    w1b6 = wbf["w1"][:, :, None].broadcast_to([128, NB, 64])
    DV.drain()
    DV.wait_ge(io.sem, pout_io)  # don't clobber props mid-DMA
    DV.tensor_tensor(t64f(), prop1(), w0b6, op=AOP.mult)
    GP.wait_ge(dv.sem, wb_dv)
    GP.wait_ge(io.sem, pout_io)
    gp.inc(GP.tensor_tensor(tmp2[:, :, 0:64], prop2(), w1b6, op=AOP.mult))
    DV.drain()
    DV.wait_ge(gp.sem, gp.n)
    dv.inc(DV.tensor_tensor(t64f(), t64f(), tmp2[:, :, 0:64], op=AOP.add))
    SY.wait_ge(dv.sem, dv.n)
    io.inc(SY.dma_start(out_o[:].rearrange("(t p) f -> p t f", p=128),
                        agg1[:, :, 0:NCLASS]))
    SY.wait_ge(io.sem, io.n)

    nc.compile()
    ctx.close()
    return nc


def _run(inputs, sim=False):
    S = inputs["x1a"].shape[0] // NCORES
    NB = -(-S // 128)
    SP = NB * 128
    NROWS = NCORES * SP

    adj = {}
    adjmeta = {}
    for a in (1, 2):
        out, meta = _prep_adjacency(
            inputs[f"src{a}"], inputs[f"dst{a}"], inputs[f"ew{a}"],
            S, SP, NB, NROWS)
        adj[a] = out
        adjmeta[a] = meta

    scalars = (float(np.asarray(inputs["g1b"]).ravel()[0]),
               float(np.asarray(inputs["g2b"]).ravel()[0]),
               float(np.asarray(inputs["h1b"]).ravel()[0]),
               float(np.asarray(inputs["h2b"]).ravel()[0]))
    nc = _build(S, SP, NB, NROWS, adjmeta, scalars)

    bf = ml_dtypes.bfloat16
    f32 = np.float32

    def wfmt(w):  # [256, 64] -> [128, 2, 64] bf16
        return np.ascontiguousarray(
            np.asarray(w, f32).reshape(2, 128, NHID).transpose(1, 0, 2)).astype(bf)

    w2pad = np.zeros((128, 64), f32)
    w2pad[:, :NCLASS] = np.asarray(inputs["W2"], f32)
    iota = np.tile(np.arange(128, dtype=f32), (128, 1))
    ident = np.eye(128, dtype=f32)
    g1w = np.tile(np.asarray(inputs["g1w"], f32).ravel(), (128, 1))
    g2w = np.tile(np.asarray(inputs["g2w"], f32).ravel(), (128, 1))
    h1w = np.zeros((128, 64), f32)
    h1w[:, :NCLASS] = np.asarray(inputs["h1w"], f32).ravel()
    h2w = np.zeros((128, 64), f32)
    h2w[:, :NCLASS] = np.asarray(inputs["h2w"], f32).ravel()
    b1r = np.tile(np.concatenate([np.asarray(inputs["b1a"], f32).ravel(),
                                  np.asarray(inputs["b1b"], f32).ravel()]), (128, 1))
    b2r = np.zeros((128, 64), f32)
    b2r[:, :NCLASS] = np.asarray(inputs["b2"], f32).ravel()

    common = dict(
        w1a=wfmt(inputs["W1a"]), w1b=wfmt(inputs["W1b"]),
        w2=w2pad.astype(bf), iota=iota.astype(bf), idf=ident.astype(bf),
        g1w=g1w.astype(bf), g2w=g2w.astype(bf),
        h1w=h1w.astype(bf), h2w=h2w.astype(bf),
        b1r=b1r.astype(bf), b2r=b2r.astype(bf))

    def xfmt(x, k):  # shard k, pad, transpose -> [128, 2, SP] bf16
        xs = np.asarray(x, f32)[k * S:(k + 1) * S]
        xp = np.zeros((SP, NFEAT), f32)
        xp[:S] = xs
        xt = xp.T.reshape(2, 128, SP).transpose(1, 0, 2)
        return np.ascontiguousarray(xt).astype(bf)

    in_maps = []
    for k in range(NCORES):
        m = dict(common)
        for v, key in (("xt1a", "x1a"), ("xt1b", "x1b"),
                       ("xt2a", "x2a"), ("xt2b", "x2b")):
            m[v] = xfmt(inputs[key], k)
        for a in (1, 2):
            g, d, e = adj[a][k]
            m[f"gidx{a}"] = g
            m[f"dst{a}"] = d
            m[f"eww{a}"] = e
        in_maps.append(m)

    global LAST_EXEC_NS
    if sim:
        from concourse.bass_interp import MultiCoreSim
        msim = MultiCoreSim(nc, NCORES)
        for k in range(NCORES):
            for name, arr in in_maps[k].items():
                msim.cores[k].tensor(name)[:] = arr
        msim.simulate()
        results = [{nm: msim.cores[k].tensor(nm).copy()
                    for nm in ("out_o", "p1_o", "p2_o")} for k in range(NCORES)]
    else:
        import os
        r = run_bass_kernel_spmd(nc, in_maps, list(range(NCORES)))
        LAST_EXEC_NS = r.exec_time_ns
        results = r.results

    outs = []
    for nm in ("out_o", "p1_o", "p2_o"):
        outs.append(np.concatenate([results[k][nm][:S] for k in range(NCORES)],
                    axis=0).astype(np.float32))
    return tuple(outs)


LAST_EXEC_NS = None


def kernel(**inputs):
    return _run(inputs, sim=False)


# revision 6
# speedup vs baseline: 1.1086x; 1.0725x over previous
"""DaGCN on 8 Trainium2 NeuronCores (Bass SPMD) — v2.

Changes vs v1 (1472us cost-model):
  * ONE merged AllGather for the L1 tables: t12 = [s1|s2] rows [SP, 256]
    bf16 -> out 25.7MB @ ~285us (vs 2x252us), exploiting the collective
    cost model's concave bandwidth ramp.
  * L2 table collective gathers the PACKED [SP, 32] bf16 shard (NCLASS=32;
    the upper 32 cols were always zero) -> out 3.2MB @ ~96us vs 252us.
    L2 gathers index NODE QUADS (row//4) and select the node via the
    256B-aligned gather offset (4-way src-row split); L1 gathers index
    NODE PAIRS (row//2, 2-way split).  The L2 idx arrays reload into the
    L1 arrays' SBUF space under the cc3 collective.
  * Phase A computes the s-tables node-major directly (no transposes),
    x loads split in halves so PE starts early; idx loads hide under the
    first collective.
  * PSUM evacuations go to ACT (both parity runs), folded with one wide
    bf16 DVE add per pass; gating math runs in bf16 split DVE/GPSIMD.
  * The L2-table transpose+matmul chain is batched 7 dst-blocks per psum
    tile so cross-engine semaphore latency amortizes (30us -> ~10us); the
    phase-A matmul loop likewise batches 4 node-slices per psum tile with
    evac copies alternating DVE/ACT, runs halves-major so each t12
    row-half DMAs while the other half computes, and defers all non-W
    constant loads under the first collective.
  * Edge meta (gidx/dst/ew) stays SBUF-resident for all 4 passes.
"""

import math
from contextlib import ExitStack

import ml_dtypes
import numpy as np

import concourse.bacc as bacc
import concourse.bass as bass
import concourse.mybir as mybir
from concourse.bass_utils import run_bass_kernel_spmd

F32 = mybir.dt.float32
BF16 = mybir.dt.bfloat16
I16 = mybir.dt.int16
AOP = mybir.AluOpType
ACT = mybir.ActivationFunctionType

NCORES = 8
N = 50000
NFEAT, NHID, NCLASS = 256, 64, 32
S_CALL = 1024          # idxs per dma_gather call (HW-validated; 2048 hangs)
CALL_CHUNKS = S_CALL // 128
RING = 16              # gather/onehot ring depth (in calls)
NPSUM = 4              # psum block-accumulator ring (L1)


def _wrap16(a):
    """[n] int16 -> [128, n//16]: idx i at [i%16, i//16], replicated x8."""
    n = a.shape[0]
    w = a.reshape(n // 16, 16).T.astype(np.int16)
    return np.tile(w, (8, 1)).copy()


def _chunkwrap(a, dtype):
    """[n] -> [128, n//128]: edge i at [i%128, i//128]."""
    n = a.shape[0]
    return np.ascontiguousarray(a.reshape(n // 128, 128).T.astype(dtype))


def _prep_adjacency(src, dst, ew, S, SP, NB, NROWS, groups=2):
    """Bucket edges by (dst core, src-row % groups, dst block).

    Returns per-core (gidx, dcol, eww) arrays plus the shared compile-time
    schedule: cpb[g][b] chunks per (group g, block b), identical across
    cores (max), per-group chunk counts padded to CALL_CHUNKS.
    """
    src = np.asarray(src).astype(np.int64)
    dst = np.asarray(dst).astype(np.int64)
    ew = np.asarray(ew).astype(np.float32)
    core = dst // S
    row = (src // S) * SP + (src % S)       # padded table row
    par = row % groups
    pair = row // groups
    dstrel = dst - core * S
    blk = dstrel // 128
    col = dstrel % 128

    percore = []
    counts = np.zeros((NCORES, groups, NB), np.int64)
    for k in range(NCORES):
        m = core == k
        e = np.lexsort((blk[m], par[m]))   # sort by (group, block)
        r, h, b, c, w = pair[m][e], par[m][e], blk[m][e], col[m][e], ew[m][e]
        percore.append((r, h, b, c, w))
        for p in range(groups):
            mm = h == p
            counts[k, p] = np.bincount(b[mm], minlength=NB)

    cpb = np.maximum(np.ceil(counts.max(axis=0) / 128).astype(np.int64), 1)
    ch = [int(cpb[p].sum()) for p in range(groups)]
    chp = [-(-c // CALL_CHUNKS) * CALL_CHUNKS for c in ch]
    nslot = sum(chp) * 128

    # chunk offset of (g, b); pads extend block NB-1's run of each group
    coff = np.zeros((groups, NB), np.int64)
    base = 0
    for p in range(groups):
        coff[p] = base + np.concatenate(([0], np.cumsum(cpb[p])))[:-1]
        base += chp[p]

    out = []
    for k in range(NCORES):
        r, h, b, c, w = percore[k]
        gidx = np.zeros(nslot, np.int64)
        dcol = np.zeros(nslot, np.int64)
        eww = np.zeros(nslot, np.float32)
        for p in range(groups):
            mm = h == p
            rr, bb, cc, ww = r[mm], b[mm], c[mm], w[mm]
            cnt = counts[k, p]
            offs = np.concatenate(([0], np.cumsum(cnt)))[:-1]
            pos = np.arange(rr.shape[0]) - offs[bb]
            slot = (coff[p][bb]) * 128 + pos
            gidx[slot] = rr
            dcol[slot] = cc
            eww[slot] = ww
        out.append((
            _wrap16(gidx),
            _chunkwrap(dcol, np.float32),
            _chunkwrap(eww, np.float32),
        ))
    return out, ([cpb[p].tolist() for p in range(groups)], chp, nslot)


def _sched_chunks(meta, NB):
    """Per global chunk: (group, block, start, stop). Pads extend the last
    block's run of their group."""
    cpbs_all, chp_all, _ = meta
    sched = []
    for p in range(len(cpbs_all)):
        cpbs, chp = cpbs_all[p], chp_all[p]
        for b in range(NB):
            for i in range(cpbs[b]):
                sched.append([p, b, i == 0, False])
        for _ in range(chp - sum(cpbs)):
            sched.append([p, NB - 1, False, False])
    last = {}
    for i, (p, b, st, sp) in enumerate(sched):
        last[(p, b)] = i
    for (p, b), i in last.items():
        sched[i][3] = True
    return sched


class Ctr:
    def __init__(self, sem, step=1):
        self.sem, self.n, self.step = sem, 0, step

    def inc(self, inst):
        inst.then_inc(self.sem, self.step)
        self.n += self.step
        return self.n


def _build(S, SP, NB, NROWS, adjmeta, adjmeta2, scalars):
    nc = bacc.Bacc("TRN2", num_devices=NCORES, num_swdge_queues=2)
    g1b, g2b, h1b, h2b = scalars
    NPAIR = NROWS // 2

    # ---------------- I/O ----------------
    din = {}
    for v in ("xt1a", "xt1b", "xt2a", "xt2b"):
        din[v] = nc.dram_tensor(v, [128, 2, SP], BF16, kind="ExternalInput")
    din["w1a"] = nc.dram_tensor("w1a", [128, 2, NHID], BF16, kind="ExternalInput")
    din["w1b"] = nc.dram_tensor("w1b", [128, 2, NHID], BF16, kind="ExternalInput")
    din["w2"] = nc.dram_tensor("w2", [128, 64], BF16, kind="ExternalInput")
    din["iota"] = nc.dram_tensor("iota", [128, 128], BF16, kind="ExternalInput")
    din["idf"] = nc.dram_tensor("idf", [128, 128], BF16, kind="ExternalInput")
    din["g1w"] = nc.dram_tensor("g1w", [128, 128], BF16, kind="ExternalInput")
    din["g2w"] = nc.dram_tensor("g2w", [128, 128], BF16, kind="ExternalInput")
    din["h1w"] = nc.dram_tensor("h1w", [128, 64], BF16, kind="ExternalInput")
    din["h2w"] = nc.dram_tensor("h2w", [128, 64], BF16, kind="ExternalInput")
    din["b1r"] = nc.dram_tensor("b1r", [128, 128], BF16, kind="ExternalInput")
    din["b2r"] = nc.dram_tensor("b2r", [128, 64], BF16, kind="ExternalInput")
    for a in (1, 2):
        ns = adjmeta[a][2]
        ns2 = adjmeta2[a][2]
        din[f"gidx{a}"] = nc.dram_tensor(f"gidx{a}", [128, ns // 16], I16, kind="ExternalInput")
        din[f"dst{a}"] = nc.dram_tensor(f"dst{a}", [128, ns // 128], F32, kind="ExternalInput")
        din[f"eww{a}"] = nc.dram_tensor(f"eww{a}", [128, ns // 128], F32, kind="ExternalInput")
        din[f"gidx3{a}"] = nc.dram_tensor(f"gidx3{a}", [128, ns2 // 16], I16, kind="ExternalInput")
        din[f"dst3{a}"] = nc.dram_tensor(f"dst3{a}", [128, ns2 // 128], F32, kind="ExternalInput")
        din[f"eww3{a}"] = nc.dram_tensor(f"eww3{a}", [128, ns2 // 128], F32, kind="ExternalInput")
    out_o = nc.dram_tensor("out_o", [SP, NCLASS], BF16, kind="ExternalOutput")
    p1_o = nc.dram_tensor("p1_o", [SP, NCLASS], BF16, kind="ExternalOutput")
    p2_o = nc.dram_tensor("p2_o", [SP, NCLASS], BF16, kind="ExternalOutput")

    t12_in = nc.dram_tensor("t12in", [SP, 256], BF16)
    t12_full = nc.dram_tensor("t12full", [NROWS, 256], BF16, addr_space="Shared")
    t3_in = nc.dram_tensor("t3in", [SP, NCLASS], BF16)
    # quad-packed: row q = nodes 4q..4q+3 x 32 cols; +1 pad row for the
    # offset gather views
    NQUAD = NROWS // 4
    t3_full = nc.dram_tensor("t3full", [NQUAD + 1, 128], BF16, addr_space="Shared")

    ctx = ExitStack()
    sb = lambda name, shape, dt: ctx.enter_context(nc.sbuf_tensor(name, shape, dt))
    ps = lambda name, shape: ctx.enter_context(nc.psum_tensor(name, shape, F32))
    sem = lambda name: ctx.enter_context(nc.semaphore(name))

    # ---------------- SBUF ----------------
    c_w1a = sb("c_w1a", [128, 2, NHID], BF16)
    c_w1b = sb("c_w1b", [128, 2, NHID], BF16)
    c_w2 = sb("c_w2", [128, 64], BF16)
    c_iota = sb("c_iota", [128, 128], BF16)
    c_idf = sb("c_idf", [128, 128], BF16)
    c_g1w = sb("c_g1w", [128, 128], BF16)
    c_g2w = sb("c_g2w", [128, 128], BF16)
    c_h1w = sb("c_h1w", [128, 64], BF16)
    c_h2w = sb("c_h2w", [128, 64], BF16)
    c_b1r = sb("c_b1r", [128, 128], BF16)
    c_b2r = sb("c_b2r", [128, 64], BF16)
    cbias = sb("cbias", [128, 4], F32)

    tstage = sb("tstage", [128, NB, 256], BF16)
    nsm = {a: max(adjmeta[a][2], adjmeta2[a][2]) for a in (1, 2)}
    gidx_sb = {a: sb(f"gidx{a}_sb", [128, nsm[a] // 16], I16) for a in (1, 2)}
    dst_sb = {a: sb(f"dst{a}_sb", [128, nsm[a] // 128], F32) for a in (1, 2)}
    ew_sb = {a: sb(f"ew{a}_sb", [128, nsm[a] // 128], F32) for a in (1, 2)}
    lamv = {nm: sb(nm, [128, NB], F32)
            for nm in ("l1", "l2", "lsum", "w0", "w1")}
    wbf = {nm: sb(f"wb_{nm}", [128, NB], BF16) for nm in ("w0", "w1")}
    xtt = sb("xtt", [128, 128], BF16)
    xttg = [sb(f"xttg{i}", [128, 896], BF16) for i in range(2)]

    sbA = ExitStack()
    xts = {v: sbA.enter_context(nc.sbuf_tensor(f"x{v}", [128, 2, SP], BF16))
           for v in ("xt1a", "xt1b", "xt2a", "xt2b")}

    psA = ExitStack()
    mm_ps = [psA.enter_context(nc.psum_tensor(f"mm_ps{i}", [128, 4, 128], F32))
             for i in range(4)]

    io = Ctr(sem("io"), 16)        # sync-engine DMAs
    gsems = [Ctr(sem(f"g{i}"), 16) for i in range(RING)]
    ccs = [Ctr(sem(f"cc{i}"), 1) for i in range(2)]
    pe = Ctr(sem("pe"), 1)
    dv = Ctr(sem("dv"), 1)
    ac = Ctr(sem("ac"), 1)
    gp = Ctr(sem("gp"), 1)

    SY, PE, DV, AC, GP = nc.sync, nc.tensor, nc.vector, nc.scalar, nc.gpsimd

    def fence():
        SY.wait_ge(io.sem, io.n)

    # =========== Phase A: constants + node-major s tables ===========
    for bi, bval in enumerate((g1b, g2b, h1b, h2b)):
        nc.vector.memset(cbias[:, bi:bi + 1], float(bval))
    dv.inc(DV.memset(xtt[:], 0))
    for name, t in (("w1a", c_w1a), ("w1b", c_w1b)):
        io.inc(SY.dma_start(t[:], din[name][:]))
    HB = NB // 2 + 1          # node-slice halves for load/compute overlap
    halves = [(0, min(HB * 128, SP))]
    if HB * 128 < SP:
        halves.append((HB * 128, SP))
    xload = {}
    for hi, (o0, o1) in enumerate(halves):
        for vi, (va, vb) in enumerate((("xt1a", "xt1b"), ("xt2a", "xt2b"))):
            io.inc(SY.dma_start(xts[va][:, :, o0:o1], din[va][:, :, o0:o1]))
            io.inc(SY.dma_start(xts[vb][:, :, o0:o1], din[vb][:, :, o0:o1]))
            fence()
            xload[(vi, hi)] = io.n
    if len(halves) == 1:
        for vi in range(2):
            xload[(vi, 1)] = xload[(vi, 0)]

    # t12 row n = [s1a[n] | s1b[n] | s2a[n] | s2b[n]], 256 bf16 cols.
    # Batch 4 node-slices per psum tile (4-deep ring) with wide evac copies
    # alternating DVE/ACT so the copy sem round-trip amortizes over 4 slices.
    GA = 4
    h0_blocks = min(HB, NB)
    segs = [(0, h0_blocks)]
    if h0_blocks < NB:
        segs.append((h0_blocks, NB))
    stc = {}
    gctr = 0
    for hi, (b0, b1) in enumerate(segs):
        for vi, (va, vb) in enumerate((("xt1a", "xt1b"), ("xt2a", "xt2b"))):
            PE.wait_ge(io.sem, xload[(vi, hi)])
            j0 = b0
            while j0 < b1:
                nj = min(GA, b1 - j0)
                p = mm_ps[gctr % 4]
                if gctr >= 4:
                    eng, val = stc[gctr - 4]
                    PE.wait_ge(dv.sem if eng == "dv" else ac.sem, val)
                last = None
                for i in range(nj):
                    o = (j0 + i) * 128
                    for bi, (xv, w) in enumerate(((va, c_w1a), (vb, c_w1b))):
                        base = bi * 64
                        for cch in range(2):
                            last = PE.matmul(p[:, i, base:base + 64],
                                             xts[xv][:, cch, o:o + 128],
                                             w[:, cch, :],
                                             start=(cch == 0), stop=(cch == 1))
                pe.inc(last)
                dst = tstage[:, j0:j0 + nj, vi * 128:vi * 128 + 128]
                if gctr % 2 == 0:
                    DV.wait_ge(pe.sem, pe.n)
                    dv.inc(DV.tensor_copy(dst, p[:, 0:nj, :]))
                    stc[gctr] = ("dv", dv.n)
                else:
                    AC.wait_ge(pe.sem, pe.n)
                    ac.inc(AC.activation(dst, p[:, 0:nj, :], ACT.Copy))
                    stc[gctr] = ("ac", ac.n)
                gctr += 1
                j0 += nj
        # both views done for this node range: ship its t12 rows now
        SY.wait_ge(dv.sem, dv.n)
        SY.wait_ge(ac.sem, ac.n)
        io.inc(SY.dma_start(
            t12_in[b0 * 128:b1 * 128, :].rearrange("(t p) f -> p t f", p=128),
            tstage[:, b0:b1, :]))
        fence()
    t12_io = io.n
    pe_phaseA = pe.n
    GP.wait_ge(io.sem, t12_io)
    ccs[0].inc(GP.collective_compute(
        "AllGather", AOP.bypass, replica_groups=[list(range(NCORES))],
        ins=[t12_in[:]], outs=[t12_full[:]]))
    # consts, the t3 pad row, and idx loads all hide under the collective
    SY.wait_ge(dv.sem, 1)   # xtt memset (first dv op)
    io.inc(SY.dma_start(t3_full[NQUAD:NQUAD + 1, :], xtt[0:1, :]))
    for name, t in (("w2", c_w2), ("iota", c_iota), ("idf", c_idf),
                    ("g1w", c_g1w), ("g2w", c_g2w), ("h1w", c_h1w),
                    ("h2w", c_h2w), ("b1r", c_b1r), ("b2r", c_b2r)):
        io.inc(SY.dma_start(t[:], din[name][:]))
    for a in (1, 2):
        ns = adjmeta[a][2]
        io.inc(SY.dma_start(gidx_sb[a][:, 0:ns // 16], din[f"gidx{a}"][:]))
        io.inc(SY.dma_start(dst_sb[a][:, 0:ns // 128], din[f"dst{a}"][:]))
        io.inc(SY.dma_start(ew_sb[a][:, 0:ns // 128], din[f"eww{a}"][:]))
    fence()
    idx_io = io.n

    # =========== edge pass machinery ===========
    psA.close()
    sbA.close()
    psL1 = ExitStack()
    blk_ps = [psL1.enter_context(nc.psum_tensor(f"blk_ps{i}", [128, 128], F32))
              for i in range(NPSUM)]
    tr2_ps = [psL1.enter_context(nc.psum_tensor(f"tr2_ps{i}", [128, 7, 64], F32))
              for i in range(2)]
    trb_ps = [psL1.enter_context(nc.psum_tensor(f"trb_ps{i}", [128, 896], BF16))
              for i in range(2)]
    msg = sb("msg", [128, RING * CALL_CHUNKS, 128], BF16)
    ohr = sb("ohr", [128, RING * CALL_CHUNKS, 128], BF16)
    agg1 = sb("agg1", [128, NB, 128], BF16)
    agg2 = sb("agg2", [128, NB, 128], BF16)
    tmp = sb("tmp", [128, NB, 128], BF16)
    tmp2 = sb("tmp2", [128, NB, 128], BF16)
    # L2 outputs alias agg2 (free after the gated combine); scratch aliases agg1
    prop1 = lambda b=None: agg2[:, :, 0:32] if b is None else agg2[:, b, 0:32]
    prop2 = lambda b=None: agg2[:, :, 32:64] if b is None else agg2[:, b, 32:64]
    t64f = lambda b=None: agg1[:, :, 0:32] if b is None else agg1[:, b, 0:32]

    scheds = {(a, 1): _sched_chunks(adjmeta[a], NB) for a in (1, 2)}
    scheds.update({(a, 2): _sched_chunks(adjmeta2[a], NB) for a in (1, 2)})
    gcall = [0]
    pe_cons_vals = []
    npass = [0]
    idx3_io = [0]
    psum_last = {}   # psum key -> (eng, val) of evac freeing it

    # gather views
    v12 = t12_full[:].rearrange("(a b) f -> a (b f)", b=2)      # [NPAIR, 512]
    t3flat = t3_full[:].rearrange("a f -> (a f)")
    l2v = {0: t3_full[0:NQUAD, :]}
    for g in (1, 2, 3):
        l2v[g] = t3flat[g * 32:g * 32 + NQUAD * 128].rearrange(
            "(a f) -> a f", f=128)

    def edge_pass(adj, layer, dest, scrs, dest_full, scr_fulls,
                  fold_eng_gp=False):
        """scrs[s-1](b) receives group-s runs; folded into dest at pass end."""
        fold_eng = GP if fold_eng_gp else DV
        meta = adjmeta[adj] if layer == 1 else adjmeta2[adj]
        sched = scheds[(adj, layer)]
        cpbs_all, chp_all, ns = meta
        if layer == 1:
            inap = {p: v12[:, (adj - 1) * 128 + p * 256:
                           (adj - 1) * 128 + p * 256 + 128] for p in (0, 1)}
            step = 512
            F = 128
            idxw = idx_io
        else:
            inap = l2v
            step = 128
            F = 32
            idxw = idx3_io[0]
        first = npass[0] == 0
        npass[0] += 1
        cc_need = 0 if layer == 1 else 1
        GP.wait_ge(ccs[cc_need].sem, 1)
        if first:
            DV.wait_ge(pe.sem, pe_phaseA)
        ch0 = 0
        firstcall = True
        for p in range(len(chp_all)):
            chp = chp_all[p]
            for j in range(chp // CALL_CHUNKS):
                rj = (gcall[0] % RING) * CALL_CHUNKS
                gslot = gcall[0] % RING
                cbase = ch0 + j * CALL_CHUNKS
                if firstcall:
                    GP.wait_ge(io.sem, idxw)
                if len(pe_cons_vals) >= RING:
                    GP.wait_ge(pe.sem, pe_cons_vals[-RING])
                g = GP.dma_gather(
                    msg[:, rj:rj + CALL_CHUNKS, :], inap[p],
                    gidx_sb[adj][:, cbase * 8:(cbase + CALL_CHUNKS) * 8],
                    S_CALL, S_CALL, 128, elem_step=step,
                    queue_num=gcall[0] % 2)
                gsems[gslot].inc(g)
                gv = gsems[gslot].n
                if firstcall:
                    DV.wait_ge(io.sem, idxw)
                    firstcall = False
                if len(pe_cons_vals) >= RING:
                    DV.wait_ge(pe.sem, pe_cons_vals[-RING])
                for c8 in range(CALL_CHUNKS):
                    ts = DV.tensor_scalar(
                        ohr[:, rj + c8, :], c_iota[:],
                        dst_sb[adj][:, cbase + c8:cbase + c8 + 1],
                        ew_sb[adj][:, cbase + c8:cbase + c8 + 1],
                        op0=AOP.is_equal, op1=AOP.mult)
                dv.inc(ts)
                ohv = dv.n
                PE.wait_ge(gsems[gslot].sem, gv)
                PE.wait_ge(dv.sem, ohv)
                for c8 in range(CALL_CHUNKS):
                    pp, b, st, sp = sched[cbase + c8]
                    key = b % NPSUM
                    ptile = blk_ps[key][:, 0:F]
                    if st and key in psum_last:
                        eng, val = psum_last[key]
                        PE.wait_ge({"dv": dv.sem, "ac": ac.sem}[eng], val)
                    mmi = PE.matmul(ptile, ohr[:, rj + c8, :],
                                    msg[:, rj + c8, 0:F],
                                    start=st, stop=sp)
                    if sp:
                        pe.inc(mmi)
                        AC.wait_ge(pe.sem, pe.n)
                        cpi = AC.activation(dest(b) if pp == 0 else scrs[pp - 1](b),
                                            ptile, ACT.Copy)
                        ac.inc(cpi)
                        psum_last[key] = ("ac", ac.n)
                if not sp:
                    pe.inc(mmi)
                pe_cons_vals.append(pe.n)
                gcall[0] += 1
            ch0 += chp
        # fold the partial-group runs into dest (wide bf16 adds)
        if fold_eng is DV:
            DV.wait_ge(ac.sem, ac.n)
            for sf in scr_fulls:
                DV.drain()
                dv.inc(DV.tensor_tensor(dest_full, dest_full, sf, op=AOP.add))
        else:
            GP.wait_ge(ac.sem, ac.n)
            for sf in scr_fulls:
                GP.tensor_tensor(dest_full, dest_full, sf, op=AOP.add)
                GP.drain()

    edge_pass(1, 1, lambda b: agg1[:, b, :], [lambda b: tmp[:, b, :]],
              agg1[:], [tmp[:]])
    edge_pass(2, 1, lambda b: agg2[:, b, :], [lambda b: tmp2[:, b, :]],
              agg2[:], [tmp2[:]], fold_eng_gp=True)

    # =========== Phase C: mid gating + L2 table ===========
    # branch 1 on DVE, branch 2 on GPSIMD (Pool idle here), all bf16
    b1b = c_b1r[:, None, :].broadcast_to([128, NB, 128])
    g1b_b = c_g1w[:, None, :].broadcast_to([128, NB, 128])
    g2b_b = c_g2w[:, None, :].broadcast_to([128, NB, 128])
    DV.drain()
    DV.tensor_tensor(agg1[:], agg1[:], b1b, op=AOP.add)
    DV.drain()
    DV.tensor_scalar(agg1[:], agg1[:], 0.0, None, op0=AOP.max)
    DV.drain()
    DV.tensor_tensor(tmp[:], agg1[:], g1b_b, op=AOP.mult)
    DV.drain()
    dv.inc(DV.tensor_reduce(lamv["l1"][:], tmp[:], axis=mybir.AxisListType.X,
                            op=AOP.add))
    l1_dv = dv.n
    GP.tensor_tensor(agg2[:], agg2[:], b1b, op=AOP.add)
    GP.drain()
    GP.tensor_scalar(agg2[:], agg2[:], 0.0, None, op0=AOP.max)
    GP.drain()
    gp.inc(GP.tensor_tensor(tmp2[:], agg2[:], g2b_b, op=AOP.mult))
    DV.wait_ge(gp.sem, gp.n)
    dv.inc(DV.tensor_reduce(lamv["l2"][:], tmp2[:], axis=mybir.AxisListType.X,
                            op=AOP.add))
    l2_dv = dv.n
    AC.wait_ge(dv.sem, l2_dv)
    AC.activation(lamv["l1"][:], lamv["l1"][:], ACT.Sigmoid, bias=cbias[:, 0:1])
    ac.inc(AC.activation(lamv["l2"][:], lamv["l2"][:], ACT.Sigmoid, bias=cbias[:, 1:2]))
    DV.wait_ge(ac.sem, ac.n)
    DV.tensor_tensor(lamv["lsum"][:], lamv["l1"][:], lamv["l2"][:], op=AOP.add)
    DV.drain()
    DV.tensor_scalar(lamv["lsum"][:], lamv["lsum"][:], 1e-12, None, op0=AOP.max)
    DV.drain()
    DV.reciprocal(lamv["lsum"][:], lamv["lsum"][:])
    DV.drain()
    DV.tensor_tensor(lamv["w0"][:], lamv["l1"][:], lamv["lsum"][:], op=AOP.mult)
    DV.tensor_tensor(lamv["w1"][:], lamv["l2"][:], lamv["lsum"][:], op=AOP.mult)
    DV.drain()
    DV.tensor_copy(wbf["w0"][:], lamv["w0"][:])
    dv.inc(DV.tensor_copy(wbf["w1"][:], lamv["w1"][:]))
    wb_dv = dv.n
    w0b = wbf["w0"][:, :, None].broadcast_to([128, NB, 128])
    w1b_ = wbf["w1"][:, :, None].broadcast_to([128, NB, 128])
    DV.drain()
    DV.tensor_tensor(agg1[:], agg1[:], w0b, op=AOP.mult)
    GP.wait_ge(dv.sem, wb_dv)
    gp.inc(GP.tensor_tensor(tmp2[:], agg2[:], w1b_, op=AOP.mult))
    DV.drain()
    DV.wait_ge(gp.sem, gp.n)
    dv.inc(DV.tensor_tensor(agg1[:], agg1[:], tmp2[:], op=AOP.add))  # x -> agg1
    xfin = dv.n


    def _s2_mm(g):
        b0 = g * GB
        nb_g = min(GB, NB - b0)
        pf = tr2_ps[g % 2]
        PE.wait_ge(dv.sem, xttc[g])
        if g >= 2:
            PE.wait_ge(dv.sem, stc2[g - 2])   # tstage copy freeing pf
        for i in range(nb_g):
            pe.inc(PE.matmul(pf[:, i, 0:32],
                             xttg[g % 2][:, i * 128:(i + 1) * 128],
                             c_w2[:, 0:32], start=True, stop=True))
        DV.wait_ge(pe.sem, pe.n)
        dv.inc(DV.tensor_copy(tstage[:, b0:b0 + nb_g, 0:32],
                              pf[:, 0:nb_g, 0:32]))
        stc2[g] = dv.n
    # L2 table: s2 = x @ W2, batched 4 blocks per psum tile so the
    # transpose->copy->matmul chain amortizes sem latency over 4 blocks
    GB = 7
    ngrp = -(-NB // GB)
    stc2 = {}
    xttc = {}
    for g in range(ngrp):
        b0 = g * GB
        nb_g = min(GB, NB - b0)
        pb = trb_ps[g % 2]
        if g == 0:
            PE.wait_ge(dv.sem, xfin)
        if g >= 2:
            PE.wait_ge(dv.sem, xttc[g - 2])   # xttg slot free (copied out)
        for i in range(nb_g):
            pe.inc(PE.transpose(pb[:, i * 128:(i + 1) * 128],
                                agg1[:, b0 + i, :], c_idf[:]))
        DV.wait_ge(pe.sem, pe.n)
        dv.inc(DV.tensor_copy(xttg[g % 2][:, 0:nb_g * 128], pb[:, 0:nb_g * 128]))
        xttc[g] = dv.n
        if g >= 1:
            _s2_mm(g - 1)
    _s2_mm(ngrp - 1)
    SY.wait_ge(dv.sem, dv.n)
    io.inc(SY.dma_start(t3_in[:].rearrange("(t p) f -> p t f", p=128),
                        tstage[:, :, 0:32]))
    fence()
    t3_io = io.n
    GP.wait_ge(io.sem, t3_io)
    ccs[1].inc(GP.collective_compute(
        "AllGather", AOP.bypass, replica_groups=[list(range(NCORES))],
        ins=[t3_in[:]],
        outs=[t3_full[0:NQUAD, :].rearrange("a (b f) -> (a b) f", b=4)]))
    # reload the idx arrays with the L2 quad-split layout (hides under cc3);
    # the last L1 gathers/onehots have long consumed them by now (pass
    # boundaries drained via the gating phase above)
    for a in (1, 2):
        ns2 = adjmeta2[a][2]
        io.inc(SY.dma_start(gidx_sb[a][:, 0:ns2 // 16], din[f"gidx3{a}"][:]))
        io.inc(SY.dma_start(dst_sb[a][:, 0:ns2 // 128], din[f"dst3{a}"][:]))
        io.inc(SY.dma_start(ew_sb[a][:, 0:ns2 // 128], din[f"eww3{a}"][:]))
    fence()
    idx3_io[0] = io.n

    # =========== L2 edge passes ===========
    edge_pass(1, 2, lambda b: prop1(b),
              [lambda b, s=s: tmp[:, b, s * 32:s * 32 + 32] for s in range(3)],
              prop1(), [tmp[:, :, s * 32:s * 32 + 32] for s in range(3)])
    sc2 = [lambda b: tmp2[:, b, 64:96], lambda b: tmp2[:, b, 96:128],
           lambda b: tmp[:, b, 96:128]]
    sf2 = [tmp2[:, :, 64:96], tmp2[:, :, 96:128], tmp[:, :, 96:128]]
    edge_pass(2, 2, lambda b: prop2(b), sc2, prop2(), sf2, fold_eng_gp=True)
    psL1.close()

    # =========== Phase F: final gating + outputs ===========
    b2b = c_b2r[:, None, 0:32].broadcast_to([128, NB, 32])
    h1b_b = c_h1w[:, None, 0:32].broadcast_to([128, NB, 32])
    h2b_b = c_h2w[:, None, 0:32].broadcast_to([128, NB, 32])
    DV.drain()
    DV.tensor_tensor(prop1(), prop1(), b2b, op=AOP.add)
    DV.drain()
    dv.inc(DV.tensor_tensor(t64f(), prop1(), h1b_b, op=AOP.mult))
    DV.drain()
    dv.inc(DV.tensor_reduce(lamv["l1"][:], t64f(), axis=mybir.AxisListType.X,
                            op=AOP.add))
    l1_dv = dv.n
    GP.tensor_tensor(prop2(), prop2(), b2b, op=AOP.add)
    GP.drain()
    gp.inc(GP.tensor_tensor(tmp2[:, :, 0:32], prop2(), h2b_b, op=AOP.mult))
    DV.wait_ge(gp.sem, gp.n)
    dv.inc(DV.tensor_reduce(lamv["l2"][:], tmp2[:, :, 0:32],
                            axis=mybir.AxisListType.X, op=AOP.add))
    l2_dv = dv.n
    SY.wait_ge(dv.sem, l2_dv)
    SY.wait_ge(gp.sem, gp.n)
    io.inc(SY.dma_start(p1_o[:].rearrange("(t p) f -> p t f", p=128),
                        agg2[:, :, 0:NCLASS]))
    io.inc(SY.dma_start(p2_o[:].rearrange("(t p) f -> p t f", p=128),
                        agg2[:, :, 32:32 + NCLASS]))
    fence()
    pout_io = io.n
    AC.wait_ge(dv.sem, l2_dv)
    AC.activation(lamv["l1"][:], lamv["l1"][:], ACT.Sigmoid, bias=cbias[:, 2:3])
    ac.inc(AC.activation(lamv["l2"][:], lamv["l2"][:], ACT.Sigmoid, bias=cbias[:, 3:4]))
    DV.wait_ge(ac.sem, ac.n)
    DV.tensor_tensor(lamv["lsum"][:], lamv["l1"][:], lamv["l2"][:], op=AOP.add)
    DV.drain()
    DV.tensor_scalar(lamv["lsum"][:], lamv["lsum"][:], 1e-12, None, op0=AOP.max)
    DV.drain()
    DV.reciprocal(lamv["lsum"][:], lamv["lsum"][:])
    DV.drain()
    DV.tensor_tensor(lamv["w0"][:], lamv["l1"][:], lamv["lsum"][:], op=AOP.mult)
    DV.tensor_tensor(lamv["w1"][:], lamv["l2"][:], lamv["lsum"][:], op=AOP.mult)
    DV.drain()
    DV.tensor_copy(wbf["w0"][:], lamv["w0"][:])
    dv.inc(DV.tensor_copy(wbf["w1"][:], lamv["w1"][:]))
    wb_dv = dv.n
    w0b6 = wbf["w0"][:, :, None].broadcast_to([128, NB, 32])
    w1b6 = wbf["w1"][:, :, None].broadcast_to([128, NB, 32])
    DV.drain()
    DV.wait_ge(io.sem, pout_io)  # don't clobber props mid-DMA
    DV.tensor_tensor(t64f(), prop1(), w0b6, op=AOP.mult)
    GP.wait_ge(dv.sem, wb_dv)
    GP.wait_ge(io.sem, pout_io)
    gp.inc(GP.tensor_tensor(tmp2[:, :, 0:32], prop2(), w1b6, op=AOP.mult))
    DV.drain()
    DV.wait_ge(gp.sem, gp.n)
    dv.inc(DV.tensor_tensor(t64f(), t64f(), tmp2[:, :, 0:32], op=AOP.add))
    SY.wait_ge(dv.sem, dv.n)
    io.inc(SY.dma_start(out_o[:].rearrange("(t p) f -> p t f", p=128),
                        agg1[:, :, 0:NCLASS]))
    SY.wait_ge(io.sem, io.n)

    nc.compile()
    ctx.close()
    return nc


def _run(inputs, sim=False):
    S = inputs["x1a"].shape[0] // NCORES
    NB = -(-S // 128)
    SP = NB * 128
    NROWS = NCORES * SP

    adj = {}
    adjmeta = {}
    adj2 = {}
    adjmeta2 = {}
    for a in (1, 2):
        out, meta = _prep_adjacency(
            inputs[f"src{a}"], inputs[f"dst{a}"], inputs[f"ew{a}"],
            S, SP, NB, NROWS, groups=2)
        adj[a] = out
        adjmeta[a] = meta
        out2, meta2 = _prep_adjacency(
            inputs[f"src{a}"], inputs[f"dst{a}"], inputs[f"ew{a}"],
            S, SP, NB, NROWS, groups=4)
        adj2[a] = out2
        adjmeta2[a] = meta2

    scalars = (float(np.asarray(inputs["g1b"]).ravel()[0]),
               float(np.asarray(inputs["g2b"]).ravel()[0]),
               float(np.asarray(inputs["h1b"]).ravel()[0]),
               float(np.asarray(inputs["h2b"]).ravel()[0]))
    nc = _build(S, SP, NB, NROWS, adjmeta, adjmeta2, scalars)

    bf = ml_dtypes.bfloat16
    f32 = np.float32

    def wfmt(w):  # [256, 64] -> [128, 2, 64] bf16
        return np.ascontiguousarray(
            np.asarray(w, f32).reshape(2, 128, NHID).transpose(1, 0, 2)).astype(bf)

    w2pad = np.zeros((128, 64), f32)
    w2pad[:, :NCLASS] = np.asarray(inputs["W2"], f32)
    iota = np.tile(np.arange(128, dtype=f32), (128, 1))
    ident = np.eye(128, dtype=f32)
    g1w = np.tile(np.asarray(inputs["g1w"], f32).ravel(), (128, 1))
    g2w = np.tile(np.asarray(inputs["g2w"], f32).ravel(), (128, 1))
    h1w = np.zeros((128, 64), f32)
    h1w[:, :NCLASS] = np.asarray(inputs["h1w"], f32).ravel()
    h2w = np.zeros((128, 64), f32)
    h2w[:, :NCLASS] = np.asarray(inputs["h2w"], f32).ravel()
    b1r = np.tile(np.concatenate([np.asarray(inputs["b1a"], f32).ravel(),
                                  np.asarray(inputs["b1b"], f32).ravel()]), (128, 1))
    b2r = np.zeros((128, 64), f32)
    b2r[:, :NCLASS] = np.asarray(inputs["b2"], f32).ravel()

    common = dict(
        w1a=wfmt(inputs["W1a"]), w1b=wfmt(inputs["W1b"]),
        w2=w2pad.astype(bf), iota=iota.astype(bf), idf=ident.astype(bf),
        g1w=g1w.astype(bf), g2w=g2w.astype(bf),
        h1w=h1w.astype(bf), h2w=h2w.astype(bf),
        b1r=b1r.astype(bf), b2r=b2r.astype(bf))

    def xfmt(x, k):  # shard k, pad, transpose -> [128, 2, SP] bf16
        xs = np.asarray(x, f32)[k * S:(k + 1) * S]
        xp = np.zeros((SP, NFEAT), f32)
        xp[:S] = xs
        xt = xp.T.reshape(2, 128, SP).transpose(1, 0, 2)
        return np.ascontiguousarray(xt).astype(bf)

    in_maps = []
    for k in range(NCORES):
        m = dict(common)
        for v, key in (("xt1a", "x1a"), ("xt1b", "x1b"),
                       ("xt2a", "x2a"), ("xt2b", "x2b")):
            m[v] = xfmt(inputs[key], k)
        for a in (1, 2):
            g, d, e = adj[a][k]
            m[f"gidx{a}"] = g
            m[f"dst{a}"] = d
            m[f"eww{a}"] = e
            g2, d2, e2 = adj2[a][k]
            m[f"gidx3{a}"] = g2
            m[f"dst3{a}"] = d2
            m[f"eww3{a}"] = e2
        in_maps.append(m)

    global LAST_EXEC_NS
    if sim:
        from concourse.bass_interp import MultiCoreSim
        msim = MultiCoreSim(nc, NCORES)
        for k in range(NCORES):
            for name, arr in in_maps[k].items():
                msim.cores[k].tensor(name)[:] = arr
        msim.simulate()
        results = [{nm: msim.cores[k].tensor(nm).copy()
                    for nm in ("out_o", "p1_o", "p2_o")} for k in range(NCORES)]
    else:
        import os
        r = run_bass_kernel_spmd(nc, in_maps, list(range(NCORES)))
        LAST_EXEC_NS = r.exec_time_ns
        results = r.results

    outs = []
    for nm in ("out_o", "p1_o", "p2_o"):
        outs.append(np.concatenate([results[k][nm][:S] for k in range(NCORES)],
                    axis=0).astype(np.float32))
    return tuple(outs)


LAST_EXEC_NS = None


def kernel(**inputs):
    return _run(inputs, sim=False)
